# revision 1
# baseline (speedup 1.0000x reference)
"""Deformable Conv2d (offset-conv -> bilinear sample -> 3x3 conv) on 8 NeuronCores.

Sharding: batch(4) x H-halves(2) -> 8 cores. Each core computes a [64, 64, 128]
slice of the output for one image. Inputs per core: a zero-padded halo slice of
its image plus (replicated) weights and index-offset constants.

Per-core device pipeline:
  1. offset conv (PE matmuls, fp16) -> offsets [18, 8192]
  2. transpose offsets to pixel-partitioned layout [128(j), 64(i), 18(ch)]
  3. index math on DVE: sampling positions, floor/frac, gather indices (int16),
     bilinear corner weight products (fp16)
  4. build a y-pair-expanded, channel-minor gather table in DRAM
     (cast to fp16 + PE transposes + 2 interleaved DMA writes)
  5. dma_gather (Pool/SWDGE): one 512B descriptor per (tap, output pixel)
     fetches all 4 bilinear corners for all 64 channels
  6. weight the gathered corners on DVE (per-pixel weights broadcast over
     channels via a step-0 free dim)
  7. PE transposes (PSUM-accumulated over the x-corner pair) to put (y-corner,
     channel) on partitions
  8. deform conv: PE matmuls contracting (y-corner, channel) per tap,
     accumulating the 9 taps in PSUM; bias via ACT on eviction.
"""

import numpy as np
from contextlib import ExitStack

B, C, H, W, O = 4, 64, 128, 128, 64
K2, CH = 9, 18
NI = 64               # output rows per core
HALO = 4
RH, RW = 72, 136      # halo slice dims (rows [h*64-4, h*64+68), cols [-4, 132))
NPIX = RH * RW        # 9792
TCH = 77              # ceil(NPIX/128) transpose chunks for the gather table
XHF = TCH * 128       # 9856 padded pixel count
TROWS = XHF           # gather-table rows (one per padded pixel)
Q = NI * W            # 8192 output pixels per core
ICH = 16              # i-rows per main-loop chunk
NCHUNK = NI // ICH    # 4 chunks
NIDX = ICH * W        # 2048 gather indices per (tap, chunk)
YCL = 70.99
XCL = 134.99

_cache = {}


def _ch_perm(ch):
    # offset-conv output channel order: ch in [0,9) -> oy of tap ch,
    # ch in [9,18) -> ox of tap ch-9. Source channel in w_off layout:
    return 2 * ch if ch < 9 else 2 * (ch - 9) + 1


def _build_consts(b_off):
    """Host-side constant tensors (identical for every core). The offset-conv
    bias is folded in here (cadd is added to the raw conv output)."""
    # cadd[j, i*18+ch]: base sampling position in halo-local coords + b_off
    cadd = np.zeros((128, NI, CH), dtype=np.float32)
    for chn in range(CH):
        if chn < 9:
            kh = chn // 3
            cadd[:, :, chn] = (np.arange(NI, dtype=np.float32) + 3 + kh)[None, :]
        else:
            kw = (chn - 9) % 3
            cadd[:, :, chn] = (np.arange(128, dtype=np.float32) + 3 + kw)[:, None]
        cadd[:, :, chn] += b_off[_ch_perm(chn)]
    return cadd.reshape(128, NI * CH)


def _prep_weights(w_off, b_off, w_dcn, b_dcn):
    # Offset-conv lhsT, packed for double-tap contraction: for each kernel row
    # kh, taps (kh,0) and (kh,1) contract together over K=128 (the image copy
    # on partitions 64-127 is pre-shifted one column), tap (kh,2) is a K=64
    # single. woffp[kh]: [128, 18]; woffs[kh]: [64, 18].
    woffp = np.zeros((3, 2 * C, CH), dtype=np.float32)
    woffs = np.zeros((3, C, CH), dtype=np.float32)
    for kh in range(3):
        for chn in range(CH):
            woffp[kh, :C, chn] = w_off[_ch_perm(chn), :, kh, 0]
            woffp[kh, C:, chn] = w_off[_ch_perm(chn), :, kh, 1]
            woffs[kh, :, chn] = w_off[_ch_perm(chn), :, kh, 2]
    # wdcn_r[k, a*64+c, o] : lhsT for deform conv tap k, replicated over the
    # y-corner index a (the transposed sampled tensor has (a, c) on partitions)
    wdcn_r = np.zeros((K2, 2 * C, O), dtype=np.float32)
    for k in range(K2):
        kh, kw = k // 3, k % 3
        wdcn_r[k, :C, :] = w_dcn[:, :, kh, kw].T
        wdcn_r[k, C:, :] = w_dcn[:, :, kh, kw].T
    return woffp, woffs, wdcn_r, b_dcn.reshape(O, 1).astype(np.float32)


def build_tile_kernel(nc, ins, out_ap, stage=99, repeat=1):
    """Emit the per-core program. ins: dict name -> AP (DRAM).
    stage truncates the pipeline for debugging (99 = full)."""
    import concourse.bass as bass
    import concourse.mybir as mybir
    import concourse.tile as tile
    from concourse.masks import make_identity

    f32 = mybir.dt.float32
    f16 = mybir.dt.float16
    i16 = mybir.dt.int16
    AF = mybir.ActivationFunctionType
    AO = mybir.AluOpType

    xi_d = ins["xi"]          # [64, NPIX] f32 halo slice
    woffp_d = ins["woffp"]    # [3, 128, 18] f32
    woffs_d = ins["woffs"]    # [3, 64, 18] f32
    wdcn_d = ins["wdcn_r"]    # [9, 128, 64] f32
    bdcn_d = ins["bdcn"]      # [64, 1] f32
    cadd_d = ins["cadd"]      # [128, 1152] f32

    tab_d = nc.dram_tensor("gtab", [(TROWS + 1) * 128], f16, kind="Internal")

    from concourse import library_config

    with ExitStack() as outer:
        tc = outer.enter_context(tile.TileContext(nc))
        nc.gpsimd.load_library(library_config.mlp)
        for _rep in range(repeat):
          with ExitStack() as ctx:
            consts = ctx.enter_context(tc.tile_pool(name="consts", bufs=1))
            sb = ctx.enter_context(tc.tile_pool(name="sb", bufs=1))
            setup_ctx = ctx.enter_context(ExitStack())
            ps_small = setup_ctx.enter_context(
                tc.tile_pool(name="ps_sm", bufs=2, space="PSUM")
            )

            # ---- constants in SBUF
            ident16 = consts.tile([128, 128], f16)
            make_identity(nc, ident16)
            ident32 = consts.tile([128, 128], f32)
            make_identity(nc, ident32)
            cadd_sb = consts.tile([128, NI * CH], f32)
            nc.sync.dma_start(cadd_sb[:], cadd_d[:])
            bdcn_sb = consts.tile([O, 1], f32)
            nc.sync.dma_start(bdcn_sb[:], bdcn_d[:])
            woffp32 = consts.tile([2 * C, 3 * CH], f32)
            nc.sync.dma_start(
                woffp32[:].rearrange("p (t c) -> p t c", t=3),
                woffp_d[:].rearrange("t p c -> p t c"),
            )
            woffph = consts.tile([2 * C, 3 * CH], f16)
            nc.vector.tensor_copy(woffph[:], woffp32[:])
            woffs32 = consts.tile([C, 3 * CH], f32)
            nc.sync.dma_start(
                woffs32[:].rearrange("p (t c) -> p t c", t=3),
                woffs_d[:].rearrange("t p c -> p t c"),
            )
            woffsh = consts.tile([C, 3 * CH], f16)
            nc.vector.tensor_copy(woffsh[:], woffs32[:])
            wdcn32 = consts.tile([128, K2 * O], f32)
            nc.sync.dma_start(
                wdcn32[:].rearrange("p (t c) -> p t c", t=K2),
                wdcn_d[:].rearrange("t p c -> p t c"),
            )
            wdcnh = consts.tile([128, K2 * O], f16)
            nc.vector.tensor_copy(wdcnh[:], wdcn32[:])

            # ---- load + cast x
            sbA = setup_ctx.enter_context(tc.tile_pool(name="sbA", bufs=1))
            sbB = setup_ctx.enter_context(tc.tile_pool(name="sbB", bufs=1))
            sbC = setup_ctx.enter_context(tc.tile_pool(name="sbC", bufs=1))
            xh = sbA.tile([128, XHF], f16)
            nc.vector.memset(xh[:, NPIX:], 0.0)
            with tc.tile_pool(name="xload", bufs=2) as xload:
                xcs = NPIX // 8  # 1224
                for t in range(8):
                    x32 = xload.tile([C, xcs], f32, tag="xc")
                    nc.sync.dma_start(x32[:], xi_d[:, t * xcs : (t + 1) * xcs])
                    nc.scalar.copy(xh[:C, t * xcs : (t + 1) * xcs], x32[:])
            # partitions 64-127: same image shifted one column left (for the
            # double-tap conv contraction)
            nc.sync.dma_start(xh[C : 2 * C, 0 : NPIX - 1], xh[0:C, 1:NPIX])
            nc.vector.memset(xh[C : 2 * C, NPIX - 1 : NPIX], 0.0)

            # ---- gather-table build: transpose to pixel-major, write twice
            xt = sbA.tile([128, TCH * C], f16)
            for t in range(TCH):
                pst = ps_small.tile([128, C], f16, tag="tabT")
                nc.tensor.transpose(
                    pst[:], xh[:C, t * 128 : (t + 1) * 128], ident16[:C, :C]
                )
                if t % 2 == 0:
                    nc.scalar.copy(xt[:, t * C : (t + 1) * C], pst[:])
                else:
                    nc.vector.tensor_copy(xt[:, t * C : (t + 1) * C], pst[:])
            xt3 = xt[:].rearrange("p (t c) -> p t c", t=TCH)
            # zero the tail rows the interleaved writes below don't fully cover
            zt = consts.tile([128, 137], f16)
            nc.vector.memset(zt[:], 0.0)
            nc.sync.dma_start(
                bass.AP(tensor=tab_d, offset=9720 * 128, ap=[[137, 128], [1, 137]]),
                zt[:],
            )
            # write A: tab[p, 0:64] = pixel p   (p = t*128 + p')
            destA = bass.AP(tensor=tab_d, offset=0, ap=[[128, 128], [128 * 128, TCH], [1, C]])
            nc.sync.dma_start(destA, xt3)
            # write B: tab[p-136, 64:128] = pixel p  (split by alignment)
            destB1 = bass.AP(
                tensor=tab_d,
                offset=(2 * 128 - 136) * 128 + 64,
                ap=[[128, 128], [128 * 128, TCH - 2], [1, C]],
            )
            nc.sync.dma_start(destB1, xt3[:, 2:TCH, :])
            destB2 = bass.AP(tensor=tab_d, offset=64, ap=[[128, 120], [1, C]])
            nc.sync.dma_start(destB2, xt3[8:128, 1, :])

            if stage < 2:
                return
            # ---- offset conv -> offs_sb [18, 8192] f32
            xh3 = xh[:, :NPIX].rearrange("p (r s) -> p r s", s=RW)
            offs_sb = sbB.tile([CH, Q], f16)
            for u in range(16):
                psc = ps_small.tile([CH, 512], f32, tag="conv")
                for kh in range(3):
                    rows = slice(u * 4 + kh + 3, u * 4 + kh + 7)
                    nc.tensor.matmul(
                        psc[:],
                        woffph[:, kh * CH : (kh + 1) * CH],
                        xh3[:, rows, 3:131],
                        start=(kh == 0),
                        stop=False,
                    )
                    nc.tensor.matmul(
                        psc[:],
                        woffsh[:, kh * CH : (kh + 1) * CH],
                        xh3[:C, rows, 5:133],
                        start=False,
                        stop=(kh == 2),
                    )
                nc.scalar.copy(offs_sb[:, u * 512 : (u + 1) * 512], psc[:])

            if stage < 3:
                return
            # ---- transpose offsets to [128(j), (i, ch)]
            offsT = sbC.tile([128, NI * CH], f32)
            for t in range(NI):
                pso = ps_small.tile([128, CH], f16, tag="offT")
                nc.tensor.transpose(
                    pso[:], offs_sb[:, t * 128 : (t + 1) * 128], ident16[:CH, :CH]
                )
                if t % 2 == 0:
                    nc.scalar.copy(offsT[:, t * CH : (t + 1) * CH], pso[:])
                else:
                    nc.vector.tensor_copy(offsT[:, t * CH : (t + 1) * CH], pso[:])

            # ---- index math (DVE) in [128, (i, ch)] layout
            pp = sbC.tile([128, NI * CH], f32)
            nc.vector.tensor_tensor(pp[:], offsT[:], cadd_sb[:], AO.add)
            nc.vector.tensor_scalar_max(pp[:], pp[:], 0.0)
            pp3 = pp[:].rearrange("p (i c) -> p i c", c=CH)
            nc.vector.tensor_scalar_min(pp3[:, :, 0:9], pp3[:, :, 0:9], YCL)
            nc.vector.tensor_scalar_min(pp3[:, :, 9:18], pp3[:, :, 9:18], XCL)
            # exact floor for 0 <= x < 2^22: magic-add rounds to nearest int,
            # then subtract 1 where the rounded value exceeds x
            MAGIC = float(1 << 23)
            fl = sbC.tile([128, NI * CH], f32)
            nc.vector.tensor_scalar(fl[:], pp[:], MAGIC, MAGIC, AO.add, AO.subtract)
            gt = sbC.tile([128, NI * CH], f32)
            nc.vector.tensor_tensor(gt[:], fl[:], pp[:], AO.is_gt)
            nc.vector.tensor_tensor(fl[:], fl[:], gt[:], AO.subtract)
            fr = gt  # reuse
            nc.vector.tensor_tensor(fr[:], pp[:], fl[:], AO.subtract)
            fl3 = fl[:].rearrange("p (i c) -> p i c", c=CH)
            idxf = sbC.tile([128, NI * K2], f32)
            idxf3 = idxf[:].rearrange("p (k i) -> p i k", i=NI)
            nc.vector.scalar_tensor_tensor(
                idxf3, fl3[:, :, 0:9], 136.0, fl3[:, :, 9:18], AO.mult, AO.add
            )
            idx16 = sb.tile([128, NI * K2], i16)
            nc.vector.tensor_copy(idx16[:], idxf[:])
            wm1 = sbC.tile([128, NI * CH], f32)
            nc.vector.tensor_scalar(wm1[:], fr[:], -1.0, 1.0, AO.mult, AO.add)
            fr3 = fr[:].rearrange("p (i c) -> p i c", c=CH)
            wm13 = wm1[:].rearrange("p (i c) -> p i c", c=CH)
            wp = sb.tile([128, NI * K2 * 4], f16)
            wp5 = wp[:].rearrange("p (i k b a) -> p i k b a", k=K2, b=2, a=2)
            for b in range(2):
                wx = fr3[:, :, 9:18] if b else wm13[:, :, 9:18]
                for a in range(2):
                    wy = fr3[:, :, 0:9] if a else wm13[:, :, 0:9]
                    nc.vector.tensor_tensor(wp5[:, :, :, b, a], wx, wy, AO.mult)

            # ---- wrap indices for dma_gather: [16, f] replicated over 8 groups
            idxw = sb.tile([128, K2 * (Q // 16)], i16)
            idxw3 = idxw[:].rearrange("p (k f) -> p k f", k=K2)
            idx163 = idx16[:].rearrange("p (k i) -> p k i", i=NI)
            idxw4 = idxw3[:, :, :].rearrange("p k (i j) -> p k i j", j=8)
            for jj in range(8):
                nc.sync.dma_start(
                    idxw4[0:16, :, :, jj],
                    idx163[16 * jj : 16 * jj + 16, :, :],
                )
            # replicate partitions 0:16 -> 16:128 by doubling
            for g in (16, 32, 64):
                nc.sync.dma_start(idxw[g : 2 * g, :], idxw[0:g, :])

            if stage == 35:
                # debug: dump idxf and a roundtripped idx16 into the output
                idxchk = sb.tile([128, NI * K2], f32)
                nc.vector.tensor_copy(idxchk[:], idx16[:])
                d0 = bass.AP(tensor=out_ap.tensor, offset=0, ap=[[576, 128], [1, 576]])
                nc.sync.dma_start(d0, idxf[:])
                d1 = bass.AP(
                    tensor=out_ap.tensor, offset=128 * 576, ap=[[576, 128], [1, 576]]
                )
                nc.sync.dma_start(d1, idxchk[:])
                d2 = bass.AP(
                    tensor=out_ap.tensor, offset=2 * 128 * 576, ap=[[1152, 128], [1, 1152]]
                )
                nc.sync.dma_start(d2, fr[:])
                return
            if stage < 4:
                return
            # ---- main loop: gather -> weight -> transpose -> deform matmul
            setup_ctx.close()
            pmain = ctx.enter_context(tc.tile_pool(name="pmain", bufs=3))
            spool = ctx.enter_context(tc.tile_pool(name="spool", bufs=2))
            ps_t = ctx.enter_context(tc.tile_pool(name="ps_t", bufs=2, space="PSUM"))
            ps_o = ctx.enter_context(tc.tile_pool(name="ps_o", bufs=1, space="PSUM"))
            gsrc = bass.AP(tensor=tab_d, offset=0, ap=[[128, TROWS], [1, 256]])
            nchunk_run = NCHUNK if stage >= 43 else 1
            ntap_run = K2 if stage != 41 else 1
            for u in range(nchunk_run):
                sacc = spool.tile([128, K2 * ICH * 128], f16, tag="S")
                sacc4 = sacc[:].rearrange("p (k i j) -> p k i j", k=K2, i=ICH)
                for k in range(ntap_run):
                    v = pmain.tile([128, ICH * 256], f16, tag="V")
                    v3 = v[:].rearrange("p (i e) -> p i e", e=256)
                    nc.gpsimd.dma_gather(
                        v3,
                        gsrc,
                        idxw3[:, k, u * (NIDX // 16) : (u + 1) * (NIDX // 16)],
                        num_idxs=NIDX,
                        num_idxs_reg=NIDX,
                        elem_size=256,
                        elem_step=128,
                        transpose=False,
                        single_packet=False,
                        queue_num=k % 4,
                    )
                    if stage < 5:
                        continue
                    v5 = v[:].rearrange("p (i b a c) -> p i b a c", i=ICH, b=2, a=2)
                    wslice = wp5[:, u * ICH : (u + 1) * ICH, k, :, :].broadcast_to(
                        [128, ICH, 2, 2, C]
                    )
                    nc.vector.tensor_tensor(v5, v5, wslice, AO.mult)
                    # sum the x-corner pair (b) -> [128, (i, a, c)]
                    vs = pmain.tile([128, ICH * 128], f16, tag="VS")
                    vs3 = vs[:].rearrange("p (i e) -> p i e", e=128)
                    nc.vector.tensor_tensor(
                        vs3, v5[:, :, 0, :, :], v5[:, :, 1, :, :], AO.add
                    )
                    pt = ps_t.tile([128, ICH * 128], f16, tag="T")
                    for i in range(ICH):
                        nc.tensor.matmul(
                            pt[:, i * 128 : (i + 1) * 128],
                            vs3[:, i, :],
                            ident16,
                            is_transpose=True,
                            start=True,
                            stop=True,
                        )
                    nc.scalar.copy(sacc4[:, k, :, :], pt[:].rearrange("p (i j) -> p i j", j=128))
                if stage < 6:
                    continue
                psos = []
                for w in range(NIDX // 512):
                    pso_w = ps_o.tile([O, 512], f32, tag=f"out{w}", name=f"pso{w}")
                    psos.append(pso_w)
                for k in range(K2):
                    for w in range(NIDX // 512):
                        nc.tensor.matmul(
                            psos[w][:],
                            wdcnh[:, k * O : (k + 1) * O],
                            sacc[:, k * ICH * 128 + w * 512 : k * ICH * 128 + (w + 1) * 512],
                            start=(k == 0),
                            stop=(k == 8),
                        )
                with tc.tile_pool(name="ob", bufs=2) as obp:
                    for w in range(NIDX // 512):
                        ob = obp.tile([O, 512], f32, tag="ob")
                        nc.vector.tensor_scalar_add(ob[:], psos[w][:], bdcn_sb[:])
                        nc.sync.dma_start(
                            out_ap[:, u * NIDX + w * 512 : u * NIDX + (w + 1) * 512], ob[:]
                        )


def _get_program():
    if "prog" in _cache:
        return _cache["prog"]
    import concourse.bacc as bacc
    import concourse.mybir as mybir

    f32 = mybir.dt.float32
    nc = bacc.Bacc("TRN2", target_bir_lowering=False, debug=False, num_devices=8, num_swdge_queues=4)
    ins = {
        "xi": nc.dram_tensor("xi", [C, NPIX], f32, kind="ExternalInput").ap(),
        "woffp": nc.dram_tensor("woffp", [3, 2 * C, CH], f32, kind="ExternalInput").ap(),
        "woffs": nc.dram_tensor("woffs", [3, C, CH], f32, kind="ExternalInput").ap(),
        "wdcn_r": nc.dram_tensor("wdcn_r", [K2, 2 * C, O], f32, kind="ExternalInput").ap(),
        "bdcn": nc.dram_tensor("bdcn", [O, 1], f32, kind="ExternalInput").ap(),
        "cadd": nc.dram_tensor("cadd", [128, NI * CH], f32, kind="ExternalInput").ap(),
    }
    out_ap = nc.dram_tensor("out", [O, Q], f32, kind="ExternalOutput").ap()
    build_tile_kernel(nc, ins, out_ap)
    nc.compile()
    _cache["prog"] = nc
    return nc


def make_in_maps(x, w_off, b_off, w_dcn, b_dcn):
    woffp, woffs, wdcn_r, bdcn = _prep_weights(
        np.asarray(w_off), np.asarray(b_off), np.asarray(w_dcn), np.asarray(b_dcn)
    )
    cadd = _build_consts(np.asarray(b_off))
    x = np.asarray(x)
    in_maps = []
    for m in range(8):
        b, h = m // 2, m % 2
        xi = np.zeros((C, RH, RW), dtype=np.float32)
        r0 = h * NI - HALO
        rlo, rhi = max(0, -r0), min(RH, H - r0)
        xi[:, rlo:rhi, HALO : HALO + W] = x[b, :, r0 + rlo : r0 + rhi, :]
        in_maps.append(
            {
                "xi": np.ascontiguousarray(xi.reshape(C, NPIX)),
                "woffp": woffp,
                "woffs": woffs,
                "wdcn_r": wdcn_r,
                "bdcn": bdcn,
                "cadd": cadd,
            }
        )
    return in_maps


def kernel(x, w_off, b_off, w_dcn, b_dcn):
    from concourse import bass_utils

    nc = _get_program()
    in_maps = make_in_maps(x, w_off, b_off, w_dcn, b_dcn)
    res = bass_utils.run_bass_kernel_spmd(nc, in_maps, core_ids=list(range(8)))
    out = np.zeros((B, O, H, W), dtype=np.float32)
    for m in range(8):
        b, h = m // 2, m % 2
        out[b, :, h * NI : (h + 1) * NI, :] = res.results[m]["out"].reshape(O, NI, W)
    return out



# revision 6
# speedup vs baseline: 1.4324x; 1.4324x over previous
"""Deformable Conv2d (offset-conv -> bilinear sample -> 3x3 conv) on 8 NeuronCores.

Sharding: batch(4) x H-halves(2) -> 8 cores. Each core computes a [64, 64, 128]
slice of the output for one image. Inputs per core: a zero-padded halo slice of
its image plus (replicated) weights and index-offset constants.

Per-core device pipeline:
  1. offset conv (PE matmuls, fp16) -> offsets [18, 8192]
  2. transpose offsets to pixel-partitioned layout [128(j), 64(i), 18(ch)]
  3. index math on DVE: sampling positions, floor/frac, gather indices (int16),
     bilinear corner weight products (fp16)
  4. build a y-pair-expanded, channel-minor gather table in DRAM
     (cast to fp16 + PE transposes + 2 interleaved DMA writes)
  5. dma_gather (Pool/SWDGE): one 512B descriptor per (tap, output pixel)
     fetches all 4 bilinear corners for all 64 channels
  6. weight the gathered corners on DVE (per-pixel weights broadcast over
     channels via a step-0 free dim)
  7. PE transposes (PSUM-accumulated over the x-corner pair) to put (y-corner,
     channel) on partitions
  8. deform conv: PE matmuls contracting (y-corner, channel) per tap,
     accumulating the 9 taps in PSUM; bias via ACT on eviction.
"""

import numpy as np
from contextlib import ExitStack

B, C, H, W, O = 4, 64, 128, 128, 64
K2, CH = 9, 18
NI = 64               # output rows per core
HALO = 4
RH, RW = 72, 136      # halo slice dims (rows [h*64-4, h*64+68), cols [-4, 132))
NPIX = RH * RW        # 9792
TCH = 77              # ceil(NPIX/128) transpose chunks for the gather table
XHF = TCH * 128       # 9856 padded pixel count
TROWS = XHF           # gather-table rows (one per padded pixel)
Q = NI * W            # 8192 output pixels per core
ICH = 16              # i-rows per main-loop chunk
NCHUNK = NI // ICH    # 4 chunks
NIDX = ICH * W        # 2048 gather indices per (tap, chunk)
YCL = 70.99
XCL = 134.99

_cache = {}


def _ch_perm(ch):
    # offset-conv output channel order: ch in [0,9) -> oy of tap ch,
    # ch in [9,18) -> ox of tap ch-9. Source channel in w_off layout:
    return 2 * ch if ch < 9 else 2 * (ch - 9) + 1


def _build_consts(b_off):
    """Host-side constant tensors (identical for every core). The offset-conv
    bias is folded in here (cadd is added to the raw conv output)."""
    # cadd[j, i*18+ch]: base sampling position in halo-local coords + b_off
    cadd = np.zeros((128, NI, CH), dtype=np.float32)
    for chn in range(CH):
        if chn < 9:
            kh = chn // 3
            cadd[:, :, chn] = (np.arange(NI, dtype=np.float32) + 3 + kh)[None, :]
        else:
            kw = (chn - 9) % 3
            cadd[:, :, chn] = (np.arange(128, dtype=np.float32) + 3 + kw)[:, None]
        cadd[:, :, chn] += b_off[_ch_perm(chn)]
    return cadd.reshape(128, NI * CH)


def _prep_weights(w_off, b_off, w_dcn, b_dcn):
    # Offset-conv lhsT, packed for double-tap contraction: for each kernel row
    # kh, taps (kh,0) and (kh,1) contract together over K=128 (the image copy
    # on partitions 64-127 is pre-shifted one column), tap (kh,2) is a K=64
    # single. woffp[kh]: [128, 18]; woffs[kh]: [64, 18].
    woffp = np.zeros((3, 2 * C, CH), dtype=np.float32)
    woffs = np.zeros((3, C, CH), dtype=np.float32)
    for kh in range(3):
        for chn in range(CH):
            woffp[kh, :C, chn] = w_off[_ch_perm(chn), :, kh, 0]
            woffp[kh, C:, chn] = w_off[_ch_perm(chn), :, kh, 1]
            woffs[kh, :, chn] = w_off[_ch_perm(chn), :, kh, 2]
    # wdcn_r[k, a*64+c, o] : lhsT for deform conv tap k, replicated over the
    # y-corner index a (the transposed sampled tensor has (a, c) on partitions)
    wdcn_r = np.zeros((K2, 2 * C, O), dtype=np.float32)
    for k in range(K2):
        kh, kw = k // 3, k % 3
        wdcn_r[k, :C, :] = w_dcn[:, :, kh, kw].T
        wdcn_r[k, C:, :] = w_dcn[:, :, kh, kw].T
    return woffp, woffs, wdcn_r, b_dcn.reshape(O, 1).astype(np.float32)


def build_tile_kernel(nc, ins, out_ap, stage=99, repeat=1):
    """Emit the per-core program. ins: dict name -> AP (DRAM).
    stage truncates the pipeline for debugging (99 = full)."""
    import concourse.bass as bass
    import concourse.mybir as mybir
    import concourse.tile as tile
    from concourse.masks import make_identity

    f32 = mybir.dt.float32
    f16 = mybir.dt.float16
    i16 = mybir.dt.int16
    AF = mybir.ActivationFunctionType
    AO = mybir.AluOpType

    xi_d = ins["xi"]          # [64, NPIX] f32 halo slice
    woffp_d = ins["woffp"]    # [3, 128, 18] f32
    woffs_d = ins["woffs"]    # [3, 64, 18] f32
    wdcn_d = ins["wdcn_r"]    # [9, 128, 64] f32
    bdcn_d = ins["bdcn"]      # [64, 1] f32
    cadd_d = ins["cadd"]      # [128, 1152] f32

    tab_d = nc.dram_tensor("gtab", [(TROWS + 1) * 128], f16, kind="Internal")

    from concourse import library_config

    with ExitStack() as outer:
        tc = outer.enter_context(tile.TileContext(nc))
        nc.gpsimd.load_library(library_config.mlp)
        for _rep in range(repeat):
          with ExitStack() as ctx:
            consts = ctx.enter_context(tc.tile_pool(name="consts", bufs=1))
            sb = ctx.enter_context(tc.tile_pool(name="sb", bufs=1))
            setup_ctx = ctx.enter_context(ExitStack())
            ps_small = setup_ctx.enter_context(
                tc.tile_pool(name="ps_sm", bufs=2, space="PSUM")
            )

            # ---- constants in SBUF
            ident16 = consts.tile([128, 128], f16)
            make_identity(nc, ident16)
            ident32 = consts.tile([128, 128], f32)
            make_identity(nc, ident32)
            cadd_sb = consts.tile([128, NI * CH], f32)
            nc.sync.dma_start(cadd_sb[:], cadd_d[:])
            bdcn_sb = consts.tile([O, 1], f32)
            nc.sync.dma_start(bdcn_sb[:], bdcn_d[:])
            woffp32 = consts.tile([2 * C, 3 * CH], f32)
            nc.sync.dma_start(
                woffp32[:].rearrange("p (t c) -> p t c", t=3),
                woffp_d[:].rearrange("t p c -> p t c"),
            )
            woffph = consts.tile([2 * C, 3 * CH], f16)
            nc.vector.tensor_copy(woffph[:], woffp32[:])
            woffs32 = consts.tile([C, 3 * CH], f32)
            nc.sync.dma_start(
                woffs32[:].rearrange("p (t c) -> p t c", t=3),
                woffs_d[:].rearrange("t p c -> p t c"),
            )
            woffsh = consts.tile([C, 3 * CH], f16)
            nc.vector.tensor_copy(woffsh[:], woffs32[:])
            wdcn32 = consts.tile([128, K2 * O], f32)
            nc.sync.dma_start(
                wdcn32[:].rearrange("p (t c) -> p t c", t=K2),
                wdcn_d[:].rearrange("t p c -> p t c"),
            )
            wdcnh = consts.tile([128, K2 * O], f16)
            nc.vector.tensor_copy(wdcnh[:], wdcn32[:])

            # ---- load + cast x
            sbA = setup_ctx.enter_context(tc.tile_pool(name="sbA", bufs=1))
            sbB = setup_ctx.enter_context(tc.tile_pool(name="sbB", bufs=1))
            sbC = setup_ctx.enter_context(tc.tile_pool(name="sbC", bufs=1))
            xh = sbA.tile([128, XHF], f16)
            nc.vector.memset(xh[:, NPIX:], 0.0)
            with tc.tile_pool(name="xload", bufs=2) as xload:
                xcs = NPIX // 8  # 1224
                for t in range(8):
                    x32 = xload.tile([C, xcs], f32, tag="xc")
                    nc.sync.dma_start(x32[:], xi_d[:, t * xcs : (t + 1) * xcs])
                    nc.scalar.copy(xh[:C, t * xcs : (t + 1) * xcs], x32[:])
            # partitions 64-127: same image shifted one column left (for the
            # double-tap conv contraction)
            nc.sync.dma_start(xh[C : 2 * C, 0 : NPIX - 1], xh[0:C, 1:NPIX])
            nc.vector.memset(xh[C : 2 * C, NPIX - 1 : NPIX], 0.0)

            # ---- gather-table build: transpose to pixel-major, interleave the
            # vertical pair (pixel p | pixel p+136) in SBUF, then one fat
            # DRAM write with 256B-contiguous runs per table row.
            xt = sbA.tile([128, TCH * 2 * C], f16)
            xt4 = xt[:].rearrange("p (t v c) -> p t v c", t=TCH, v=2)
            for t in range(TCH):
                pst = ps_small.tile([128, C], f16, tag="tabT")
                nc.tensor.transpose(
                    pst[:], xh[:C, t * 128 : (t + 1) * 128], ident16[:C, :C]
                )
                if t % 2 == 0:
                    nc.scalar.copy(xt4[:, t, 0, :], pst[:])
                else:
                    nc.vector.tensor_copy(xt4[:, t, 0, :], pst[:])
            # pair slot: xt4[p, t, 1, :] = pixel (t*128+p)+136 = xt4[p+8, t+1, 0, :]
            # (only rows < 9656 are ever gathered; tails can hold garbage)
            nc.scalar.dma_start(xt4[0:120, 0 : TCH - 1, 1, :], xt4[8:128, 1:TCH, 0, :])
            nc.sync.dma_start(xt4[120:128, 0 : TCH - 2, 1, :], xt4[0:8, 2:TCH, 0, :])
            TH = TCH // 2
            destA = bass.AP(
                tensor=tab_d, offset=0, ap=[[128, 128], [128 * 128, TH], [1, 2 * C]]
            )
            destB = bass.AP(
                tensor=tab_d,
                offset=TH * 128 * 128,
                ap=[[128, 128], [128 * 128, TCH - TH], [1, 2 * C]],
            )
            xt3f = xt4.rearrange("p t v c -> p t (v c)")
            nc.sync.dma_start(destA, xt3f[:, :TH, :])
            nc.scalar.dma_start(destB, xt3f[:, TH:, :])

            if stage < 2:
                return
            # ---- offset conv -> offs_sb [18, 8192] f32
            xh3 = xh[:, :NPIX].rearrange("p (r s) -> p r s", s=RW)
            offs_sb = sbB.tile([CH, Q], f16)
            for u in range(16):
                psc = ps_small.tile([CH, 512], f32, tag="conv")
                for kh in range(3):
                    rows = slice(u * 4 + kh + 3, u * 4 + kh + 7)
                    nc.tensor.matmul(
                        psc[:],
                        woffph[:, kh * CH : (kh + 1) * CH],
                        xh3[:, rows, 3:131],
                        start=(kh == 0),
                        stop=False,
                    )
                    nc.tensor.matmul(
                        psc[:],
                        woffsh[:, kh * CH : (kh + 1) * CH],
                        xh3[:C, rows, 5:133],
                        start=False,
                        stop=(kh == 2),
                    )
                nc.scalar.copy(offs_sb[:, u * 512 : (u + 1) * 512], psc[:])

            if stage < 3:
                return
            # ---- transpose offsets to [128(j), (i, ch)]
            offsT = sbC.tile([128, NI * CH], f32)
            for t in range(NI):
                pso = ps_small.tile([128, CH], f16, tag="offT")
                nc.tensor.transpose(
                    pso[:], offs_sb[:, t * 128 : (t + 1) * 128], ident16[:CH, :CH]
                )
                if t % 2 == 0:
                    nc.scalar.copy(offsT[:, t * CH : (t + 1) * CH], pso[:])
                else:
                    nc.vector.tensor_copy(offsT[:, t * CH : (t + 1) * CH], pso[:])

            # ---- index math (DVE) in [128, (i, ch)] layout
            pp = sbC.tile([128, NI * CH], f32)
            nc.vector.tensor_tensor(pp[:], offsT[:], cadd_sb[:], AO.add)
            nc.vector.tensor_scalar_max(pp[:], pp[:], 0.0)
            pp3 = pp[:].rearrange("p (i c) -> p i c", c=CH)
            nc.vector.tensor_scalar_min(pp3[:, :, 0:9], pp3[:, :, 0:9], YCL)
            nc.vector.tensor_scalar_min(pp3[:, :, 9:18], pp3[:, :, 9:18], XCL)
            # exact floor for 0 <= x < 2^22: magic-add rounds to nearest int,
            # then subtract 1 where the rounded value exceeds x
            MAGIC = float(1 << 23)
            fl = sbC.tile([128, NI * CH], f32)
            nc.vector.tensor_scalar(fl[:], pp[:], MAGIC, MAGIC, AO.add, AO.subtract)
            gt = sbC.tile([128, NI * CH], f32)
            nc.vector.tensor_tensor(gt[:], fl[:], pp[:], AO.is_gt)
            nc.vector.tensor_tensor(fl[:], fl[:], gt[:], AO.subtract)
            fr = gt  # reuse
            nc.vector.tensor_tensor(fr[:], pp[:], fl[:], AO.subtract)
            fl3 = fl[:].rearrange("p (i c) -> p i c", c=CH)
            idxf = sbC.tile([128, NI * K2], f32)
            idxf3 = idxf[:].rearrange("p (k i) -> p i k", i=NI)
            nc.vector.scalar_tensor_tensor(
                idxf3, fl3[:, :, 0:9], 136.0, fl3[:, :, 9:18], AO.mult, AO.add
            )
            idx16 = sb.tile([128, NI * K2], i16)
            nc.vector.tensor_copy(idx16[:], idxf[:])
            wm1 = sbC.tile([128, NI * CH], f32)
            nc.vector.tensor_scalar(wm1[:], fr[:], -1.0, 1.0, AO.mult, AO.add)
            fr3 = fr[:].rearrange("p (i c) -> p i c", c=CH)
            wm13 = wm1[:].rearrange("p (i c) -> p i c", c=CH)
            wp = sb.tile([128, NI * K2 * 4], f16)
            wp5 = wp[:].rearrange("p (i k b a) -> p i k b a", k=K2, b=2, a=2)
            for b in range(2):
                wx = fr3[:, :, 9:18] if b else wm13[:, :, 9:18]
                for a in range(2):
                    wy = fr3[:, :, 0:9] if a else wm13[:, :, 0:9]
                    nc.vector.tensor_tensor(wp5[:, :, :, b, a], wx, wy, AO.mult)

            # ---- wrap indices for dma_gather: [16, f] replicated over 8 groups.
            # Done per tap k so the first gather only waits for its own tap's
            # wrap; the rest overlap with the main loop.
            idxw = sb.tile([128, K2 * (Q // 16)], i16)
            idxw3 = idxw[:].rearrange("p (k f) -> p k f", k=K2)
            idx163 = idx16[:].rearrange("p (k i) -> p k i", i=NI)
            idxw4 = idxw3[:, :, :].rearrange("p k (i j) -> p k i j", j=8)
            for k in range(K2):
                for jj in range(8):
                    nc.sync.dma_start(
                        idxw4[0:16, k, :, jj],
                        idx163[16 * jj : 16 * jj + 16, k, :],
                    )
                for g in (16, 32, 64):
                    nc.sync.dma_start(
                        idxw3[g : 2 * g, k, :], idxw3[0:g, k, :]
                    )

            if stage == 35:
                # debug: dump idxf and a roundtripped idx16 into the output
                idxchk = sb.tile([128, NI * K2], f32)
                nc.vector.tensor_copy(idxchk[:], idx16[:])
                d0 = bass.AP(tensor=out_ap.tensor, offset=0, ap=[[576, 128], [1, 576]])
                nc.sync.dma_start(d0, idxf[:])
                d1 = bass.AP(
                    tensor=out_ap.tensor, offset=128 * 576, ap=[[576, 128], [1, 576]]
                )
                nc.sync.dma_start(d1, idxchk[:])
                d2 = bass.AP(
                    tensor=out_ap.tensor, offset=2 * 128 * 576, ap=[[1152, 128], [1, 1152]]
                )
                nc.sync.dma_start(d2, fr[:])
                return
            if stage < 4:
                return
            # ---- main loop: gather -> weight -> transpose -> per-tap deform
            # matmul accumulated in PSUM (no sacc staging buffer).
            setup_ctx.close()
            pmain = ctx.enter_context(tc.tile_pool(name="pmain", bufs=8))
            vspool = ctx.enter_context(tc.tile_pool(name="vspool", bufs=3))
            stpool = ctx.enter_context(tc.tile_pool(name="stpool", bufs=3))
            ps_t = ctx.enter_context(tc.tile_pool(name="ps_t", bufs=2, space="PSUM"))
            ps_o = ctx.enter_context(tc.tile_pool(name="ps_o", bufs=1, space="PSUM"))
            obp = ctx.enter_context(tc.tile_pool(name="ob", bufs=2))
            gsrc = bass.AP(tensor=tab_d, offset=0, ap=[[128, TROWS], [1, 256]])
            nchunk_run = NCHUNK if stage >= 43 else 1
            ntap_run = K2 if stage != 41 else 1
            for u in range(nchunk_run):
                psos = []
                for w in range(NIDX // 512):
                    pso_w = ps_o.tile([O, 512], f32, tag=f"out{w}", name=f"pso{w}")
                    psos.append(pso_w)
                for k in range(ntap_run):
                    v = pmain.tile([128, ICH * 256], f16, tag="V")
                    v3 = v[:].rearrange("p (i e) -> p i e", e=256)
                    nc.gpsimd.dma_gather(
                        v3,
                        gsrc,
                        idxw3[:, k, u * (NIDX // 16) : (u + 1) * (NIDX // 16)],
                        num_idxs=NIDX,
                        num_idxs_reg=NIDX,
                        elem_size=256,
                        elem_step=128,
                        transpose=False,
                        single_packet=False,
                        queue_num=(u * K2 + k) % 4,
                    )
                    if stage < 5:
                        continue
                    v5 = v[:].rearrange("p (i b a c) -> p i b a c", i=ICH, b=2, a=2)
                    wslice = wp5[:, u * ICH : (u + 1) * ICH, k, :, :].broadcast_to(
                        [128, ICH, 2, 2, C]
                    )
                    nc.vector.tensor_tensor(v5, v5, wslice, AO.mult)
                    # sum the x-corner pair (b) -> [128, (i, a, c)]
                    vs = vspool.tile([128, ICH * 128], f16, tag="VS")
                    vs3 = vs[:].rearrange("p (i e) -> p i e", e=128)
                    nc.vector.tensor_tensor(
                        vs3, v5[:, :, 0, :, :], v5[:, :, 1, :, :], AO.add
                    )
                    pt = ps_t.tile([128, ICH * 128], f16, tag="T")
                    for i in range(ICH):
                        nc.tensor.matmul(
                            pt[:, i * 128 : (i + 1) * 128],
                            vs3[:, i, :],
                            ident16,
                            is_transpose=True,
                            start=True,
                            stop=True,
                        )
                    if stage < 6:
                        continue
                    stap = stpool.tile([128, ICH * 128], f16, tag="ST")
                    nc.scalar.copy(stap[:], pt[:])
                    for w in range(NIDX // 512):
                        nc.tensor.matmul(
                            psos[w][:],
                            wdcnh[:, k * O : (k + 1) * O],
                            stap[:, w * 512 : (w + 1) * 512],
                            start=(k == 0),
                            stop=(k == 8),
                        )
                if stage < 6:
                    continue
                for w in range(NIDX // 512):
                    ob = obp.tile([O, 512], f32, tag="ob")
                    nc.vector.tensor_scalar_add(ob[:], psos[w][:], bdcn_sb[:])
                    nc.sync.dma_start(
                        out_ap[:, u * NIDX + w * 512 : u * NIDX + (w + 1) * 512], ob[:]
                    )


def _get_program():
    if "prog" in _cache:
        return _cache["prog"]
    import concourse.bacc as bacc
    import concourse.mybir as mybir

    f32 = mybir.dt.float32
    nc = bacc.Bacc("TRN2", target_bir_lowering=False, debug=False, num_devices=8, num_swdge_queues=4)
    ins = {
        "xi": nc.dram_tensor("xi", [C, NPIX], f32, kind="ExternalInput").ap(),
        "woffp": nc.dram_tensor("woffp", [3, 2 * C, CH], f32, kind="ExternalInput").ap(),
        "woffs": nc.dram_tensor("woffs", [3, C, CH], f32, kind="ExternalInput").ap(),
        "wdcn_r": nc.dram_tensor("wdcn_r", [K2, 2 * C, O], f32, kind="ExternalInput").ap(),
        "bdcn": nc.dram_tensor("bdcn", [O, 1], f32, kind="ExternalInput").ap(),
        "cadd": nc.dram_tensor("cadd", [128, NI * CH], f32, kind="ExternalInput").ap(),
    }
    out_ap = nc.dram_tensor("out", [O, Q], f32, kind="ExternalOutput").ap()
    build_tile_kernel(nc, ins, out_ap)
    nc.compile()
    _cache["prog"] = nc
    return nc


def make_in_maps(x, w_off, b_off, w_dcn, b_dcn):
    woffp, woffs, wdcn_r, bdcn = _prep_weights(
        np.asarray(w_off), np.asarray(b_off), np.asarray(w_dcn), np.asarray(b_dcn)
    )
    cadd = _build_consts(np.asarray(b_off))
    x = np.asarray(x)
    in_maps = []
    for m in range(8):
        b, h = m // 2, m % 2
        xi = np.zeros((C, RH, RW), dtype=np.float32)
        r0 = h * NI - HALO
        rlo, rhi = max(0, -r0), min(RH, H - r0)
        xi[:, rlo:rhi, HALO : HALO + W] = x[b, :, r0 + rlo : r0 + rhi, :]
        in_maps.append(
            {
                "xi": np.ascontiguousarray(xi.reshape(C, NPIX)),
                "woffp": woffp,
                "woffs": woffs,
                "wdcn_r": wdcn_r,
                "bdcn": bdcn,
                "cadd": cadd,
            }
        )
    return in_maps


def kernel(x, w_off, b_off, w_dcn, b_dcn):
    from concourse import bass_utils

    nc = _get_program()
    in_maps = make_in_maps(x, w_off, b_off, w_dcn, b_dcn)
    res = bass_utils.run_bass_kernel_spmd(nc, in_maps, core_ids=list(range(8)))
    out = np.zeros((B, O, H, W), dtype=np.float32)
    for m in range(8):
        b, h = m // 2, m % 2
        out[b, :, h * NI : (h + 1) * NI, :] = res.results[m]["out"].reshape(O, NI, W)
    return out



# revision 12
# speedup vs baseline: 1.4409x; 1.0059x over previous
"""Deformable Conv2d (offset-conv -> bilinear sample -> 3x3 conv) on 8 NeuronCores.

Sharding: batch(4) x H-halves(2) -> 8 cores. Each core computes a [64, 64, 128]
slice of the output for one image. Inputs per core: a zero-padded halo slice of
its image plus (replicated) weights and index-offset constants.

Per-core device pipeline:
  1. offset conv (PE matmuls, fp16) -> offsets [18, 8192]
  2. transpose offsets to pixel-partitioned layout [128(j), 64(i), 18(ch)]
  3. index math on DVE: sampling positions, floor/frac, gather indices (int16),
     bilinear corner weight products (fp16)
  4. build a y-pair-expanded, channel-minor gather table in DRAM
     (cast to fp16 + PE transposes + 2 interleaved DMA writes)
  5. dma_gather (Pool/SWDGE): one 512B descriptor per (tap, output pixel)
     fetches all 4 bilinear corners for all 64 channels
  6. weight the gathered corners on DVE (per-pixel weights broadcast over
     channels via a step-0 free dim)
  7. PE transposes (PSUM-accumulated over the x-corner pair) to put (y-corner,
     channel) on partitions
  8. deform conv: PE matmuls contracting (y-corner, channel) per tap,
     accumulating the 9 taps in PSUM; bias via ACT on eviction.
"""

import numpy as np
from contextlib import ExitStack

B, C, H, W, O = 4, 64, 128, 128, 64
K2, CH = 9, 18
NI = 64               # output rows per core
HALO = 4
RH, RW = 72, 136      # halo slice dims (rows [h*64-4, h*64+68), cols [-4, 132))
NPIX = RH * RW        # 9792
TCH = 77              # ceil(NPIX/128) transpose chunks for the gather table
XHF = TCH * 128       # 9856 padded pixel count
TROWS = XHF           # gather-table rows (one per padded pixel)
Q = NI * W            # 8192 output pixels per core
ICH = 16              # i-rows per main-loop chunk
NCHUNK = NI // ICH    # 4 chunks
NIDX = ICH * W        # 2048 gather indices per (tap, chunk)
YCL = 70.99
XCL = 134.99

_cache = {}


def _ch_perm(ch):
    # offset-conv output channel order: ch in [0,9) -> oy of tap ch,
    # ch in [9,18) -> ox of tap ch-9. Source channel in w_off layout:
    return 2 * ch if ch < 9 else 2 * (ch - 9) + 1


def _build_consts(b_off):
    """Host-side constant tensors (identical for every core). The offset-conv
    bias is folded in here (cadd is added to the raw conv output)."""
    # cadd[j, i*18+ch]: base sampling position in halo-local coords + b_off
    cadd = np.zeros((128, NI, CH), dtype=np.float32)
    for chn in range(CH):
        if chn < 9:
            kh = chn // 3
            cadd[:, :, chn] = (np.arange(NI, dtype=np.float32) + 3 + kh)[None, :]
        else:
            kw = (chn - 9) % 3
            cadd[:, :, chn] = (np.arange(128, dtype=np.float32) + 3 + kw)[:, None]
        cadd[:, :, chn] += b_off[_ch_perm(chn)]
    return cadd.reshape(128, NI * CH)


def _prep_weights(w_off, b_off, w_dcn, b_dcn):
    # Offset-conv lhsT, packed for double-tap contraction: for each kernel row
    # kh, taps (kh,0) and (kh,1) contract together over K=128 (the image copy
    # on partitions 64-127 is pre-shifted one column), tap (kh,2) is a K=64
    # single. woffp[kh]: [128, 18]; woffs[kh]: [64, 18].
    woffp = np.zeros((3, 2 * C, CH), dtype=np.float32)
    woffs = np.zeros((3, C, CH), dtype=np.float32)
    for kh in range(3):
        for chn in range(CH):
            woffp[kh, :C, chn] = w_off[_ch_perm(chn), :, kh, 0]
            woffp[kh, C:, chn] = w_off[_ch_perm(chn), :, kh, 1]
            woffs[kh, :, chn] = w_off[_ch_perm(chn), :, kh, 2]
    # wdcn_r[k, a*64+c, o] : lhsT for deform conv tap k, replicated over the
    # y-corner index a (the transposed sampled tensor has (a, c) on partitions)
    wdcn_r = np.zeros((K2, 2 * C, O), dtype=np.float32)
    for k in range(K2):
        kh, kw = k // 3, k % 3
        wdcn_r[k, :C, :] = w_dcn[:, :, kh, kw].T
        wdcn_r[k, C:, :] = w_dcn[:, :, kh, kw].T
    return woffp, woffs, wdcn_r, b_dcn.reshape(O, 1).astype(np.float32)


def build_tile_kernel(nc, ins, out_ap, stage=99, repeat=1):
    """Emit the per-core program. ins: dict name -> AP (DRAM).
    stage truncates the pipeline for debugging (99 = full)."""
    import concourse.bass as bass
    import concourse.mybir as mybir
    import concourse.tile as tile
    from concourse.masks import make_identity

    f32 = mybir.dt.float32
    f16 = mybir.dt.float16
    i16 = mybir.dt.int16
    AF = mybir.ActivationFunctionType
    AO = mybir.AluOpType

    xi_d = ins["xi"]          # [64, NPIX] f32 halo slice
    woffp_d = ins["woffp"]    # [3, 128, 18] f32
    woffs_d = ins["woffs"]    # [3, 64, 18] f32
    wdcn_d = ins["wdcn_r"]    # [9, 128, 64] f32
    bdcn_d = ins["bdcn"]      # [64, 1] f32
    cadd_d = ins["cadd"]      # [128, 1152] f32

    tab_d = nc.dram_tensor("gtab", [(TROWS + 1) * 128], f16, kind="Internal")

    from concourse import library_config

    with ExitStack() as outer:
        tc = outer.enter_context(tile.TileContext(nc))
        nc.gpsimd.load_library(library_config.mlp)
        for _rep in range(repeat):
          with ExitStack() as ctx:
            consts = ctx.enter_context(tc.tile_pool(name="consts", bufs=1))
            sb = ctx.enter_context(tc.tile_pool(name="sb", bufs=1))
            setup_ctx = ctx.enter_context(ExitStack())
            ps_small = setup_ctx.enter_context(
                tc.tile_pool(name="ps_sm", bufs=2, space="PSUM")
            )

            # ---- constants in SBUF
            ident16 = consts.tile([128, 128], f16)
            make_identity(nc, ident16)
            ident32 = consts.tile([128, 128], f32)
            make_identity(nc, ident32)
            cadd_sb = consts.tile([128, NI * CH], f32)
            nc.sync.dma_start(cadd_sb[:], cadd_d[:])
            bdcn_sb = consts.tile([O, 1], f32)
            nc.sync.dma_start(bdcn_sb[:], bdcn_d[:])
            woffp32 = consts.tile([2 * C, 3 * CH], f32)
            nc.sync.dma_start(
                woffp32[:].rearrange("p (t c) -> p t c", t=3),
                woffp_d[:].rearrange("t p c -> p t c"),
            )
            woffph = consts.tile([2 * C, 3 * CH], f16)
            nc.vector.tensor_copy(woffph[:], woffp32[:])
            woffs32 = consts.tile([C, 3 * CH], f32)
            nc.sync.dma_start(
                woffs32[:].rearrange("p (t c) -> p t c", t=3),
                woffs_d[:].rearrange("t p c -> p t c"),
            )
            woffsh = consts.tile([C, 3 * CH], f16)
            nc.vector.tensor_copy(woffsh[:], woffs32[:])
            wdcn32 = consts.tile([128, K2 * O], f32)
            nc.sync.dma_start(
                wdcn32[:].rearrange("p (t c) -> p t c", t=K2),
                wdcn_d[:].rearrange("t p c -> p t c"),
            )
            wdcnh = consts.tile([128, K2 * O], f16)
            nc.vector.tensor_copy(wdcnh[:], wdcn32[:])

            # ---- load + cast x
            sbA = setup_ctx.enter_context(tc.tile_pool(name="sbA", bufs=1))
            sbB = setup_ctx.enter_context(tc.tile_pool(name="sbB", bufs=1))
            sbC = setup_ctx.enter_context(tc.tile_pool(name="sbC", bufs=1))
            xh = sbA.tile([128, XHF], f16)
            nc.vector.memset(xh[:, NPIX:], 0.0)
            with tc.tile_pool(name="xload", bufs=4) as xload:
                xcs = NPIX // 8  # 1224
                for t in range(8):
                    x32 = xload.tile([C, xcs], f32, tag="xc")
                    eng = nc.sync if t % 2 == 0 else nc.scalar
                    eng.dma_start(x32[:], xi_d[:, t * xcs : (t + 1) * xcs])
                    if t % 2 == 0:
                        nc.scalar.copy(xh[:C, t * xcs : (t + 1) * xcs], x32[:])
                    else:
                        nc.vector.tensor_copy(xh[:C, t * xcs : (t + 1) * xcs], x32[:])
            # partitions 64-127: same image shifted one column left (for the
            # double-tap conv contraction)
            nc.sync.dma_start(xh[C : 2 * C, 0 : NPIX - 1], xh[0:C, 1:NPIX])
            nc.vector.memset(xh[C : 2 * C, NPIX - 1 : NPIX], 0.0)

            # ---- gather-table build: transpose to pixel-major, interleave the
            # vertical pair (pixel p | pixel p+136) in SBUF, then one fat
            # DRAM write with 256B-contiguous runs per table row.
            xt = sbA.tile([128, TCH * 2 * C], f16)
            xt4 = xt[:].rearrange("p (t v c) -> p t v c", t=TCH, v=2)
            for t in range(TCH):
                pst = ps_small.tile([128, C], f16, tag="tabT")
                nc.tensor.transpose(
                    pst[:], xh[:C, t * 128 : (t + 1) * 128], ident16[:C, :C]
                )
                if t % 2 == 0:
                    nc.scalar.copy(xt4[:, t, 0, :], pst[:])
                else:
                    nc.vector.tensor_copy(xt4[:, t, 0, :], pst[:])
            # pair slot: xt4[p, t, 1, :] = pixel (t*128+p)+136 = xt4[p+8, t+1, 0, :]
            # (only rows < 9656 are ever gathered; tails can hold garbage)
            nc.scalar.dma_start(xt4[0:120, 0 : TCH - 1, 1, :], xt4[8:128, 1:TCH, 0, :])
            nc.sync.dma_start(xt4[120:128, 0 : TCH - 2, 1, :], xt4[0:8, 2:TCH, 0, :])
            TH = TCH // 2
            destA = bass.AP(
                tensor=tab_d, offset=0, ap=[[128, 128], [128 * 128, TH], [1, 2 * C]]
            )
            destB = bass.AP(
                tensor=tab_d,
                offset=TH * 128 * 128,
                ap=[[128, 128], [128 * 128, TCH - TH], [1, 2 * C]],
            )
            xt3f = xt4.rearrange("p t v c -> p t (v c)")
            nc.sync.dma_start(destA, xt3f[:, :TH, :])
            nc.scalar.dma_start(destB, xt3f[:, TH:, :])

            if stage < 2:
                return
            # ---- offset conv -> offs_sb [18, 8192] f32
            xh3 = xh[:, :NPIX].rearrange("p (r s) -> p r s", s=RW)
            offs_sb = sbB.tile([CH, Q], f16)
            for u in range(16):
                psc = ps_small.tile([CH, 512], f32, tag="conv")
                for kh in range(3):
                    rows = slice(u * 4 + kh + 3, u * 4 + kh + 7)
                    nc.tensor.matmul(
                        psc[:],
                        woffph[:, kh * CH : (kh + 1) * CH],
                        xh3[:, rows, 3:131],
                        start=(kh == 0),
                        stop=False,
                    )
                    nc.tensor.matmul(
                        psc[:],
                        woffsh[:, kh * CH : (kh + 1) * CH],
                        xh3[:C, rows, 5:133],
                        start=False,
                        stop=(kh == 2),
                    )
                nc.scalar.copy(offs_sb[:, u * 512 : (u + 1) * 512], psc[:])

            if stage < 3:
                return
            # ---- transpose offsets to [128(j), (i, ch)]
            offsT = sbC.tile([128, NI * CH], f32)
            for t in range(NI):
                pso = ps_small.tile([128, CH], f16, tag="offT")
                nc.tensor.transpose(
                    pso[:], offs_sb[:, t * 128 : (t + 1) * 128], ident16[:CH, :CH]
                )
                if t % 2 == 0:
                    nc.scalar.copy(offsT[:, t * CH : (t + 1) * CH], pso[:])
                else:
                    nc.vector.tensor_copy(offsT[:, t * CH : (t + 1) * CH], pso[:])

            # ---- index math (DVE) in [128, (i, ch)] layout
            pp = sbC.tile([128, NI * CH], f32)
            nc.vector.tensor_tensor(pp[:], offsT[:], cadd_sb[:], AO.add)
            nc.vector.tensor_scalar_max(pp[:], pp[:], 0.0)
            pp3 = pp[:].rearrange("p (i c) -> p i c", c=CH)
            nc.vector.tensor_scalar_min(pp3[:, :, 0:9], pp3[:, :, 0:9], YCL)
            nc.vector.tensor_scalar_min(pp3[:, :, 9:18], pp3[:, :, 9:18], XCL)
            # exact floor for 0 <= x < 2^22: magic-add rounds to nearest int,
            # then subtract 1 where the rounded value exceeds x
            MAGIC = float(1 << 23)
            fl = sbC.tile([128, NI * CH], f32)
            nc.vector.tensor_scalar(fl[:], pp[:], MAGIC, MAGIC, AO.add, AO.subtract)
            gt = sbC.tile([128, NI * CH], f32)
            nc.vector.tensor_tensor(gt[:], fl[:], pp[:], AO.is_gt)
            nc.vector.tensor_tensor(fl[:], fl[:], gt[:], AO.subtract)
            fr = gt  # reuse
            nc.vector.tensor_tensor(fr[:], pp[:], fl[:], AO.subtract)
            fl3 = fl[:].rearrange("p (i c) -> p i c", c=CH)
            idxf = sbC.tile([128, NI * K2], f32)
            idxf3 = idxf[:].rearrange("p (k i) -> p i k", i=NI)
            nc.vector.scalar_tensor_tensor(
                idxf3, fl3[:, :, 0:9], 136.0, fl3[:, :, 9:18], AO.mult, AO.add
            )
            idx16 = sb.tile([128, NI * K2], i16)
            nc.vector.tensor_copy(idx16[:], idxf[:])
            wm1 = sbC.tile([128, NI * CH], f32)
            nc.vector.tensor_scalar(wm1[:], fr[:], -1.0, 1.0, AO.mult, AO.add)
            fr3 = fr[:].rearrange("p (i c) -> p i c", c=CH)
            wm13 = wm1[:].rearrange("p (i c) -> p i c", c=CH)
            wp = sb.tile([128, NI * K2 * 4], f16)
            wp5 = wp[:].rearrange("p (i k b a) -> p i k b a", k=K2, b=2, a=2)
            for b in range(2):
                wx = fr3[:, :, 9:18] if b else wm13[:, :, 9:18]
                for a in range(2):
                    wy = fr3[:, :, 0:9] if a else wm13[:, :, 0:9]
                    nc.vector.tensor_tensor(wp5[:, :, :, b, a], wx, wy, AO.mult)

            # ---- wrap indices for dma_gather: [16, f] replicated over 8 groups.
            # Done per tap k so the first gather only waits for its own tap's
            # wrap; the rest overlap with the main loop.
            idxw = sb.tile([128, K2 * (Q // 16)], i16)
            idxw3 = idxw[:].rearrange("p (k f) -> p k f", k=K2)
            idx163 = idx16[:].rearrange("p (k i) -> p k i", i=NI)
            idxw4 = idxw3[:, :, :].rearrange("p k (i j) -> p k i j", j=8)
            for k in range(K2):
                for jj in range(8):
                    eng = nc.sync if jj % 2 == 0 else nc.scalar
                    eng.dma_start(
                        idxw4[0:16, k, :, jj],
                        idx163[16 * jj : 16 * jj + 16, k, :],
                    )
                for gi, g in enumerate((16, 32, 64)):
                    eng = nc.sync if gi % 2 == 0 else nc.scalar
                    eng.dma_start(idxw3[g : 2 * g, k, :], idxw3[0:g, k, :])

            if stage == 35:
                # debug: dump idxf and a roundtripped idx16 into the output
                idxchk = sb.tile([128, NI * K2], f32)
                nc.vector.tensor_copy(idxchk[:], idx16[:])
                d0 = bass.AP(tensor=out_ap.tensor, offset=0, ap=[[576, 128], [1, 576]])
                nc.sync.dma_start(d0, idxf[:])
                d1 = bass.AP(
                    tensor=out_ap.tensor, offset=128 * 576, ap=[[576, 128], [1, 576]]
                )
                nc.sync.dma_start(d1, idxchk[:])
                d2 = bass.AP(
                    tensor=out_ap.tensor, offset=2 * 128 * 576, ap=[[1152, 128], [1, 1152]]
                )
                nc.sync.dma_start(d2, fr[:])
                return
            if stage < 4:
                return
            # ---- main loop: gather -> weight -> transpose -> per-tap deform
            # matmul accumulated in PSUM (no sacc staging buffer).
            setup_ctx.close()
            pmain = ctx.enter_context(tc.tile_pool(name="pmain", bufs=8))
            vspool = ctx.enter_context(tc.tile_pool(name="vspool", bufs=3))
            stpool = ctx.enter_context(tc.tile_pool(name="stpool", bufs=3))
            ps_t = ctx.enter_context(tc.tile_pool(name="ps_t", bufs=2, space="PSUM"))
            ps_o = ctx.enter_context(tc.tile_pool(name="ps_o", bufs=1, space="PSUM"))
            obp = ctx.enter_context(tc.tile_pool(name="ob", bufs=2))
            gsrc = bass.AP(tensor=tab_d, offset=0, ap=[[128, TROWS], [1, 256]])
            nchunk_run = NCHUNK if stage >= 43 else 1
            ntap_run = K2 if stage != 41 else 1
            for u in range(nchunk_run):
                psos = []
                for w in range(NIDX // 512):
                    pso_w = ps_o.tile([O, 512], f32, tag=f"out{w}", name=f"pso{w}")
                    psos.append(pso_w)
                for k in range(ntap_run):
                    v = pmain.tile([128, ICH * 256], f16, tag="V")
                    v3 = v[:].rearrange("p (i e) -> p i e", e=256)
                    nc.gpsimd.dma_gather(
                        v3,
                        gsrc,
                        idxw3[:, k, u * (NIDX // 16) : (u + 1) * (NIDX // 16)],
                        num_idxs=NIDX,
                        num_idxs_reg=NIDX,
                        elem_size=256,
                        elem_step=128,
                        transpose=False,
                        single_packet=False,
                        queue_num=(u * K2 + k) % 4,
                    )
                    if stage < 5:
                        continue
                    v5 = v[:].rearrange("p (i b a c) -> p i b a c", i=ICH, b=2, a=2)
                    wslice = wp5[:, u * ICH : (u + 1) * ICH, k, :, :].broadcast_to(
                        [128, ICH, 2, 2, C]
                    )
                    nc.vector.tensor_tensor(v5, v5, wslice, AO.mult)
                    # sum the x-corner pair (b) -> [128, (i, a, c)]
                    vs = vspool.tile([128, ICH * 128], f16, tag="VS")
                    vs3 = vs[:].rearrange("p (i e) -> p i e", e=128)
                    nc.vector.tensor_tensor(
                        vs3, v5[:, :, 0, :, :], v5[:, :, 1, :, :], AO.add
                    )
                    pt = ps_t.tile([128, ICH * 128], f16, tag="T")
                    for i in range(ICH):
                        nc.tensor.matmul(
                            pt[:, i * 128 : (i + 1) * 128],
                            vs3[:, i, :],
                            ident16,
                            is_transpose=True,
                            start=True,
                            stop=True,
                        )
                    if stage < 6:
                        continue
                    stap = stpool.tile([128, ICH * 128], f16, tag="ST")
                    nc.scalar.copy(stap[:], pt[:])
                    for w in range(NIDX // 512):
                        nc.tensor.matmul(
                            psos[w][:],
                            wdcnh[:, k * O : (k + 1) * O],
                            stap[:, w * 512 : (w + 1) * 512],
                            start=(k == 0),
                            stop=(k == 8),
                        )
                if stage < 6:
                    continue
                for w in range(NIDX // 512):
                    ob = obp.tile([O, 512], f32, tag="ob")
                    nc.vector.tensor_scalar_add(ob[:], psos[w][:], bdcn_sb[:])
                    nc.sync.dma_start(
                        out_ap[:, u * NIDX + w * 512 : u * NIDX + (w + 1) * 512], ob[:]
                    )


def _get_program():
    if "prog" in _cache:
        return _cache["prog"]
    import concourse.bacc as bacc
    import concourse.mybir as mybir

    f32 = mybir.dt.float32
    nc = bacc.Bacc(
        "TRN2",
        target_bir_lowering=False,
        debug=False,
        num_devices=8,
        num_swdge_queues=4,
    )
    ins = {
        "xi": nc.dram_tensor("xi", [C, NPIX], f32, kind="ExternalInput").ap(),
        "woffp": nc.dram_tensor("woffp", [3, 2 * C, CH], f32, kind="ExternalInput").ap(),
        "woffs": nc.dram_tensor("woffs", [3, C, CH], f32, kind="ExternalInput").ap(),
        "wdcn_r": nc.dram_tensor("wdcn_r", [K2, 2 * C, O], f32, kind="ExternalInput").ap(),
        "bdcn": nc.dram_tensor("bdcn", [O, 1], f32, kind="ExternalInput").ap(),
        "cadd": nc.dram_tensor("cadd", [128, NI * CH], f32, kind="ExternalInput").ap(),
    }
    out_ap = nc.dram_tensor("out", [O, Q], f32, kind="ExternalOutput").ap()
    build_tile_kernel(nc, ins, out_ap)
    nc.compile()
    _cache["prog"] = nc
    return nc


def make_in_maps(x, w_off, b_off, w_dcn, b_dcn):
    woffp, woffs, wdcn_r, bdcn = _prep_weights(
        np.asarray(w_off), np.asarray(b_off), np.asarray(w_dcn), np.asarray(b_dcn)
    )
    cadd = _build_consts(np.asarray(b_off))
    x = np.asarray(x)
    in_maps = []
    for m in range(8):
        b, h = m // 2, m % 2
        xi = np.zeros((C, RH, RW), dtype=np.float32)
        r0 = h * NI - HALO
        rlo, rhi = max(0, -r0), min(RH, H - r0)
        xi[:, rlo:rhi, HALO : HALO + W] = x[b, :, r0 + rlo : r0 + rhi, :]
        in_maps.append(
            {
                "xi": np.ascontiguousarray(xi.reshape(C, NPIX)),
                "woffp": woffp,
                "woffs": woffs,
                "wdcn_r": wdcn_r,
                "bdcn": bdcn,
                "cadd": cadd,
            }
        )
    return in_maps


def kernel(x, w_off, b_off, w_dcn, b_dcn):
    from concourse import bass_utils

    nc = _get_program()
    in_maps = make_in_maps(x, w_off, b_off, w_dcn, b_dcn)
    res = bass_utils.run_bass_kernel_spmd(nc, in_maps, core_ids=list(range(8)))
    out = np.zeros((B, O, H, W), dtype=np.float32)
    for m in range(8):
        b, h = m // 2, m % 2
        out[b, :, h * NI : (h + 1) * NI, :] = res.results[m]["out"].reshape(O, NI, W)
    return out



# revision 16
# speedup vs baseline: 2.1022x; 1.4590x over previous
"""Deformable Conv2d (offset-conv -> bilinear sample -> 3x3 conv) on 8 NeuronCores.

Sharding: batch(4) x H-halves(2) -> 8 cores. Each core computes a [64, 64, 128]
slice of the output for one image. Inputs per core: a zero-padded halo slice of
its image plus (replicated) weights and index-offset constants.

Per-core device pipeline:
  1. offset conv (PE matmuls, fp16) -> offsets [18, 8192]
  2. transpose offsets to pixel-partitioned layout [128(j), 64(i), 18(ch)]
  3. index math on DVE: sampling positions, floor/frac, gather indices (int16),
     bilinear corner weight products (fp16)
  4. build a y-pair-expanded, channel-minor gather table in DRAM
     (cast to fp16 + PE transposes + 2 interleaved DMA writes)
  5. dma_gather (Pool/SWDGE): one 512B descriptor per (tap, output pixel)
     fetches all 4 bilinear corners for all 64 channels
  6. weight the gathered corners on DVE (per-pixel weights broadcast over
     channels via a step-0 free dim)
  7. PE transposes (PSUM-accumulated over the x-corner pair) to put (y-corner,
     channel) on partitions
  8. deform conv: PE matmuls contracting (y-corner, channel) per tap,
     accumulating the 9 taps in PSUM; bias via ACT on eviction.
"""

import numpy as np
from contextlib import ExitStack

B, C, H, W, O = 4, 64, 128, 128, 64
K2, CH = 9, 18
NI = 64               # output rows per core
HALO = 4
RH, RW = 72, 136      # halo slice dims (rows [h*64-4, h*64+68), cols [-4, 132))
NPIX = RH * RW        # 9792
TCH = 77              # ceil(NPIX/128) transpose chunks for the gather table
XHF = TCH * 128       # 9856 padded pixel count
TROWS = XHF           # gather-table rows (one per padded pixel)
Q = NI * W            # 8192 output pixels per core
ICH = 16              # i-rows per main-loop chunk
NCHUNK = NI // ICH    # 4 chunks
NIDX = ICH * W        # 2048 gather indices per (tap, chunk)
YCL = 70.99
XCL = 134.99

_cache = {}


def _ch_perm(ch):
    # offset-conv output channel order: ch in [0,9) -> oy of tap ch,
    # ch in [9,18) -> ox of tap ch-9. Source channel in w_off layout:
    return 2 * ch if ch < 9 else 2 * (ch - 9) + 1


def _build_consts(b_off):
    """Host-side constant tensors (identical for every core). The offset-conv
    bias is folded in here (cadd is added to the raw conv output)."""
    # cadd[j, i*18+ch]: base sampling position in halo-local coords + b_off
    cadd = np.zeros((128, NI, CH), dtype=np.float32)
    for chn in range(CH):
        if chn < 9:
            kh = chn // 3
            cadd[:, :, chn] = (np.arange(NI, dtype=np.float32) + 3 + kh)[None, :]
        else:
            kw = (chn - 9) % 3
            cadd[:, :, chn] = (np.arange(128, dtype=np.float32) + 3 + kw)[:, None]
        cadd[:, :, chn] += b_off[_ch_perm(chn)]
    return cadd.reshape(128, NI * CH)


def _prep_weights(w_off, b_off, w_dcn, b_dcn):
    # Offset-conv lhsT, packed for double-tap contraction: for each kernel row
    # kh, taps (kh,0) and (kh,1) contract together over K=128 (the image copy
    # on partitions 64-127 is pre-shifted one column), tap (kh,2) is a K=64
    # single. woffp[kh]: [128, 18]; woffs[kh]: [64, 18].
    woffp = np.zeros((3, 2 * C, CH), dtype=np.float32)
    woffs = np.zeros((3, C, CH), dtype=np.float32)
    for kh in range(3):
        for chn in range(CH):
            woffp[kh, :C, chn] = w_off[_ch_perm(chn), :, kh, 0]
            woffp[kh, C:, chn] = w_off[_ch_perm(chn), :, kh, 1]
            woffs[kh, :, chn] = w_off[_ch_perm(chn), :, kh, 2]
    # wdcn_r[k, a*64+c, o] : lhsT for deform conv tap k, replicated over the
    # y-corner index a (the transposed sampled tensor has (a, c) on partitions)
    wdcn_r = np.zeros((K2, 2 * C, O), dtype=np.float32)
    for k in range(K2):
        kh, kw = k // 3, k % 3
        wdcn_r[k, :C, :] = w_dcn[:, :, kh, kw].T
        wdcn_r[k, C:, :] = w_dcn[:, :, kh, kw].T
    return woffp, woffs, wdcn_r, b_dcn.reshape(O, 1).astype(np.float32)


def build_tile_kernel(nc, ins, out_ap, stage=99, repeat=1):
    """Emit the per-core program. ins: dict name -> AP (DRAM).
    stage truncates the pipeline for debugging (99 = full)."""
    import concourse.bass as bass
    import concourse.mybir as mybir
    import concourse.tile as tile
    from concourse.masks import make_identity

    f32 = mybir.dt.float32
    f16 = mybir.dt.float16
    i16 = mybir.dt.int16
    AF = mybir.ActivationFunctionType
    AO = mybir.AluOpType

    xi_d = ins["xi"]          # [64, NPIX] f32 halo slice
    woffp_d = ins["woffp"]    # [3, 128, 18] f32
    woffs_d = ins["woffs"]    # [3, 64, 18] f32
    wdcn_d = ins["wdcn_r"]    # [9, 128, 64] f32
    bdcn_d = ins["bdcn"]      # [64, 1] f32
    cadd_d = ins["cadd"]      # [128, 1152] f32

    tab_d = nc.dram_tensor("gtab", [(TROWS + 1) * 128], f16, kind="Internal")

    from concourse import library_config

    with ExitStack() as outer:
        tc = outer.enter_context(tile.TileContext(nc))
        nc.gpsimd.load_library(library_config.mlp)
        for _rep in range(repeat):
          with ExitStack() as ctx:
            consts = ctx.enter_context(tc.tile_pool(name="consts", bufs=1))
            sb = ctx.enter_context(tc.tile_pool(name="sb", bufs=1))
            setup_ctx = ctx.enter_context(ExitStack())
            ps_small = setup_ctx.enter_context(
                tc.tile_pool(name="ps_sm", bufs=2, space="PSUM")
            )

            # ---- constants in SBUF
            ident16 = consts.tile([128, 128], f16)
            make_identity(nc, ident16)
            ident32 = consts.tile([128, 128], f32)
            make_identity(nc, ident32)
            cadd_sb = consts.tile([128, NI * CH], f32)
            nc.sync.dma_start(cadd_sb[:], cadd_d[:])
            bdcn_sb = consts.tile([O, 1], f32)
            nc.sync.dma_start(bdcn_sb[:], bdcn_d[:])
            woffp32 = consts.tile([2 * C, 3 * CH], f32)
            nc.sync.dma_start(
                woffp32[:].rearrange("p (t c) -> p t c", t=3),
                woffp_d[:].rearrange("t p c -> p t c"),
            )
            woffph = consts.tile([2 * C, 3 * CH], f16)
            nc.vector.tensor_copy(woffph[:], woffp32[:])
            woffs32 = consts.tile([C, 3 * CH], f32)
            nc.sync.dma_start(
                woffs32[:].rearrange("p (t c) -> p t c", t=3),
                woffs_d[:].rearrange("t p c -> p t c"),
            )
            woffsh = consts.tile([C, 3 * CH], f16)
            nc.vector.tensor_copy(woffsh[:], woffs32[:])
            wdcn32 = consts.tile([128, K2 * O], f32)
            nc.sync.dma_start(
                wdcn32[:].rearrange("p (t c) -> p t c", t=K2),
                wdcn_d[:].rearrange("t p c -> p t c"),
            )
            wdcnh = consts.tile([128, K2 * O], f16)
            nc.vector.tensor_copy(wdcnh[:], wdcn32[:])

            # ---- load + cast x
            sbA = setup_ctx.enter_context(tc.tile_pool(name="sbA", bufs=1))
            sbB = setup_ctx.enter_context(tc.tile_pool(name="sbB", bufs=1))
            sbC = setup_ctx.enter_context(tc.tile_pool(name="sbC", bufs=1))
            xh = sbA.tile([128, XHF], f16)
            nc.vector.memset(xh[:, NPIX:], 0.0)
            with tc.tile_pool(name="xload", bufs=4) as xload:
                xcs = NPIX // 8  # 1224
                for t in range(8):
                    x32 = xload.tile([C, xcs], f32, tag="xc")
                    eng = nc.sync if t % 2 == 0 else nc.scalar
                    eng.dma_start(x32[:], xi_d[:, t * xcs : (t + 1) * xcs])
                    if t % 2 == 0:
                        nc.scalar.copy(xh[:C, t * xcs : (t + 1) * xcs], x32[:])
                    else:
                        nc.vector.tensor_copy(xh[:C, t * xcs : (t + 1) * xcs], x32[:])
            # partitions 64-127: same image shifted one column left (for the
            # double-tap conv contraction)
            nc.sync.dma_start(xh[C : 2 * C, 0 : NPIX - 1], xh[0:C, 1:NPIX])
            nc.vector.memset(xh[C : 2 * C, NPIX - 1 : NPIX], 0.0)

            # ---- gather-table build: transpose to pixel-major, interleave the
            # vertical pair (pixel p | pixel p+136) in SBUF, then one fat
            # DRAM write with 256B-contiguous runs per table row.
            xt = sbA.tile([128, TCH * 2 * C], f16)
            xt4 = xt[:].rearrange("p (t v c) -> p t v c", t=TCH, v=2)
            for t in range(TCH):
                pst = ps_small.tile([128, C], f16, tag="tabT")
                nc.tensor.transpose(
                    pst[:], xh[:C, t * 128 : (t + 1) * 128], ident16[:C, :C]
                )
                if t % 2 == 0:
                    nc.scalar.copy(xt4[:, t, 0, :], pst[:])
                else:
                    nc.vector.tensor_copy(xt4[:, t, 0, :], pst[:])
            # pair slot: xt4[p, t, 1, :] = pixel (t*128+p)+136 = xt4[p+8, t+1, 0, :]
            # (only rows < 9656 are ever gathered; tails can hold garbage)
            nc.scalar.dma_start(xt4[0:120, 0 : TCH - 1, 1, :], xt4[8:128, 1:TCH, 0, :])
            nc.sync.dma_start(xt4[120:128, 0 : TCH - 2, 1, :], xt4[0:8, 2:TCH, 0, :])
            TH = TCH // 2
            destA = bass.AP(
                tensor=tab_d, offset=0, ap=[[128, 128], [128 * 128, TH], [1, 2 * C]]
            )
            destB = bass.AP(
                tensor=tab_d,
                offset=TH * 128 * 128,
                ap=[[128, 128], [128 * 128, TCH - TH], [1, 2 * C]],
            )
            xt3f = xt4.rearrange("p t v c -> p t (v c)")
            nc.sync.dma_start(destA, xt3f[:, :TH, :])
            nc.scalar.dma_start(destB, xt3f[:, TH:, :])

            if stage < 2:
                return
            # ---- offset conv -> offs_sb [18, 8192] f32
            xh3 = xh[:, :NPIX].rearrange("p (r s) -> p r s", s=RW)
            offs_sb = sbB.tile([CH, Q], f16)
            for u in range(16):
                psc = ps_small.tile([CH, 512], f32, tag="conv")
                for kh in range(3):
                    rows = slice(u * 4 + kh + 3, u * 4 + kh + 7)
                    nc.tensor.matmul(
                        psc[:],
                        woffph[:, kh * CH : (kh + 1) * CH],
                        xh3[:, rows, 3:131],
                        start=(kh == 0),
                        stop=False,
                    )
                    nc.tensor.matmul(
                        psc[:],
                        woffsh[:, kh * CH : (kh + 1) * CH],
                        xh3[:C, rows, 5:133],
                        start=False,
                        stop=(kh == 2),
                    )
                nc.scalar.copy(offs_sb[:, u * 512 : (u + 1) * 512], psc[:])

            if stage < 3:
                return
            # ---- transpose offsets to [128(j), (i, ch)]
            offsT = sbC.tile([128, NI * CH], f32)
            for t in range(NI):
                pso = ps_small.tile([128, CH], f16, tag="offT")
                nc.tensor.transpose(
                    pso[:], offs_sb[:, t * 128 : (t + 1) * 128], ident16[:CH, :CH]
                )
                if t % 2 == 0:
                    nc.scalar.copy(offsT[:, t * CH : (t + 1) * CH], pso[:])
                else:
                    nc.vector.tensor_copy(offsT[:, t * CH : (t + 1) * CH], pso[:])

            # ---- index math (DVE) in [128, (i, ch)] layout
            pp = sbC.tile([128, NI * CH], f32)
            nc.vector.tensor_tensor(pp[:], offsT[:], cadd_sb[:], AO.add)
            nc.vector.tensor_scalar_max(pp[:], pp[:], 0.0)
            pp3 = pp[:].rearrange("p (i c) -> p i c", c=CH)
            nc.vector.tensor_scalar_min(pp3[:, :, 0:9], pp3[:, :, 0:9], YCL)
            nc.vector.tensor_scalar_min(pp3[:, :, 9:18], pp3[:, :, 9:18], XCL)
            # exact floor for 0 <= x < 2^22: magic-add rounds to nearest int,
            # then subtract 1 where the rounded value exceeds x
            MAGIC = float(1 << 23)
            fl = sbC.tile([128, NI * CH], f32)
            nc.vector.tensor_scalar(fl[:], pp[:], MAGIC, MAGIC, AO.add, AO.subtract)
            gt = sbC.tile([128, NI * CH], f32)
            nc.vector.tensor_tensor(gt[:], fl[:], pp[:], AO.is_gt)
            nc.vector.tensor_tensor(fl[:], fl[:], gt[:], AO.subtract)
            fr = gt  # reuse
            nc.vector.tensor_tensor(fr[:], pp[:], fl[:], AO.subtract)
            fl3 = fl[:].rearrange("p (i c) -> p i c", c=CH)
            idxf = sbC.tile([128, NI * K2], f32)
            idxf3 = idxf[:].rearrange("p (k i) -> p i k", i=NI)
            nc.vector.scalar_tensor_tensor(
                idxf3, fl3[:, :, 0:9], 136.0, fl3[:, :, 9:18], AO.mult, AO.add
            )
            idx16 = sb.tile([128, NI * K2], i16)
            nc.vector.tensor_copy(idx16[:], idxf[:])
            wm1 = sbC.tile([128, NI * CH], f32)
            nc.vector.tensor_scalar(wm1[:], fr[:], -1.0, 1.0, AO.mult, AO.add)
            fr3 = fr[:].rearrange("p (i c) -> p i c", c=CH)
            wm13 = wm1[:].rearrange("p (i c) -> p i c", c=CH)
            wp = sb.tile([128, NI * K2 * 4], f16)
            wp5 = wp[:].rearrange("p (i k b a) -> p i k b a", k=K2, b=2, a=2)
            for b in range(2):
                wx = fr3[:, :, 9:18] if b else wm13[:, :, 9:18]
                for a in range(2):
                    wy = fr3[:, :, 0:9] if a else wm13[:, :, 0:9]
                    nc.vector.tensor_tensor(wp5[:, :, :, b, a], wx, wy, AO.mult)

            # ---- wrap indices for dma_gather: [16, f] replicated over 8 groups.
            # Partition fold 128->16 via 8 bulk partition-shift DMAs into a
            # (jj, k, i) staging tile, then a per-tap DVE free-dim permute to
            # (k, i, jj); replication to 128 partitions by doubling DMAs.
            idxw = sb.tile([128, K2 * (Q // 16)], i16)
            idxw3 = idxw[:].rearrange("p (k f) -> p k f", k=K2)
            idx163 = idx16[:].rearrange("p (k i) -> p k i", i=NI)
            idxw4 = idxw3[:, :, :].rearrange("p k (i j) -> p k i j", j=8)
            tmpw = sb.tile([16, 8 * K2 * NI], i16)
            tmp3 = tmpw[:].rearrange("p (j k i) -> p j k i", j=8, k=K2)
            for jj in range(8):
                eng = nc.sync if jj % 2 == 0 else nc.scalar
                eng.dma_start(tmp3[:, jj, :, :], idx163[16 * jj : 16 * jj + 16, :, :])
            for k in range(K2):
                nc.vector.tensor_copy(
                    idxw4[0:16, k, :, :],
                    tmp3[:, :, k, :].rearrange("p j i -> p i j"),
                )
                for gi, g in enumerate((16, 32, 64)):
                    eng = nc.sync if gi % 2 == 0 else nc.scalar
                    eng.dma_start(idxw3[g : 2 * g, k, :], idxw3[0:g, k, :])

            if stage == 35:
                # debug: dump idxf and a roundtripped idx16 into the output
                idxchk = sb.tile([128, NI * K2], f32)
                nc.vector.tensor_copy(idxchk[:], idx16[:])
                d0 = bass.AP(tensor=out_ap.tensor, offset=0, ap=[[576, 128], [1, 576]])
                nc.sync.dma_start(d0, idxf[:])
                d1 = bass.AP(
                    tensor=out_ap.tensor, offset=128 * 576, ap=[[576, 128], [1, 576]]
                )
                nc.sync.dma_start(d1, idxchk[:])
                d2 = bass.AP(
                    tensor=out_ap.tensor, offset=2 * 128 * 576, ap=[[1152, 128], [1, 1152]]
                )
                nc.sync.dma_start(d2, fr[:])
                return
            if stage < 4:
                return
            # ---- main loop: gather -> weight -> transpose -> per-tap deform
            # matmul accumulated in PSUM (no sacc staging buffer).
            setup_ctx.close()
            pmain = ctx.enter_context(tc.tile_pool(name="pmain", bufs=8))
            vspool = ctx.enter_context(tc.tile_pool(name="vspool", bufs=3))
            stpool = ctx.enter_context(tc.tile_pool(name="stpool", bufs=3))
            ps_o = ctx.enter_context(tc.tile_pool(name="ps_o", bufs=2, space="PSUM"))
            obp = ctx.enter_context(tc.tile_pool(name="ob", bufs=2))
            gsrc = bass.AP(tensor=tab_d, offset=0, ap=[[128, TROWS], [1, 256]])
            nchunk_run = NCHUNK if stage >= 43 else 1
            ntap_run = K2 if stage != 41 else 1
            for u in range(nchunk_run):
                psos = []
                for w in range(NIDX // 512):
                    pso_w = ps_o.tile([O, 512], f32, tag=f"out{w}", name=f"pso{w}")
                    psos.append(pso_w)
                for k in range(ntap_run):
                    v = pmain.tile([128, ICH * 256], f16, tag="V")
                    v3 = v[:].rearrange("p (i e) -> p i e", e=256)
                    # two 1024-idx halves on different queues: smaller ring
                    # footprint avoids Pool head-of-line blocking on ring-full
                    for h in range(2):
                        nc.gpsimd.dma_gather(
                            v3[:, h * (ICH // 2) : (h + 1) * (ICH // 2), :],
                            gsrc,
                            idxw3[
                                :,
                                k,
                                u * (NIDX // 16) + h * (NIDX // 32) : u * (NIDX // 16)
                                + (h + 1) * (NIDX // 32),
                            ],
                            num_idxs=NIDX // 2,
                            num_idxs_reg=NIDX // 2,
                            elem_size=256,
                            elem_step=128,
                            transpose=False,
                            single_packet=False,
                            queue_num=(u * K2 * 2 + k * 2 + h) % 4,
                        )
                    if stage < 5:
                        continue
                    v5 = v[:].rearrange("p (i b a c) -> p i b a c", i=ICH, b=2, a=2)
                    wslice = wp5[:, u * ICH : (u + 1) * ICH, k, :, :].broadcast_to(
                        [128, ICH, 2, 2, C]
                    )
                    nc.vector.tensor_tensor(v5, v5, wslice, AO.mult)
                    # sum the x-corner pair (b) -> [128, (i, a, c)]
                    vs = vspool.tile([128, ICH * 128], f16, tag="VS")
                    vs3 = vs[:].rearrange("p (i e) -> p i e", e=128)
                    nc.vector.tensor_tensor(
                        vs3, v5[:, :, 0, :, :], v5[:, :, 1, :, :], AO.add
                    )
                    if stage < 6:
                        continue
                    # per-i-block 128x128 transpose via the DMA XBAR: frees the
                    # PE (576 transposes) and the PSUM->SBUF copies entirely
                    stap = stpool.tile([128, ICH * 128], f16, tag="ST")
                    teng = nc.sync if k % 2 == 0 else nc.scalar
                    teng.dma_start_transpose(
                        stap[:].rearrange("p (i e) -> p i e", e=128), vs[:]
                    )
                    for w in range(NIDX // 512):
                        nc.tensor.matmul(
                            psos[w][:],
                            wdcnh[:, k * O : (k + 1) * O],
                            stap[:, w * 512 : (w + 1) * 512],
                            start=(k == 0),
                            stop=(k == 8),
                        )
                if stage < 6:
                    continue
                for w in range(NIDX // 512):
                    ob = obp.tile([O, 512], f32, tag="ob")
                    nc.vector.tensor_scalar_add(ob[:], psos[w][:], bdcn_sb[:])
                    nc.sync.dma_start(
                        out_ap[:, u * NIDX + w * 512 : u * NIDX + (w + 1) * 512], ob[:]
                    )


def _get_program():
    if "prog" in _cache:
        return _cache["prog"]
    import concourse.bacc as bacc
    import concourse.mybir as mybir

    f32 = mybir.dt.float32
    nc = bacc.Bacc(
        "TRN2",
        target_bir_lowering=False,
        debug=False,
        num_devices=8,
        num_swdge_queues=4,
    )
    ins = {
        "xi": nc.dram_tensor("xi", [C, NPIX], f32, kind="ExternalInput").ap(),
        "woffp": nc.dram_tensor("woffp", [3, 2 * C, CH], f32, kind="ExternalInput").ap(),
        "woffs": nc.dram_tensor("woffs", [3, C, CH], f32, kind="ExternalInput").ap(),
        "wdcn_r": nc.dram_tensor("wdcn_r", [K2, 2 * C, O], f32, kind="ExternalInput").ap(),
        "bdcn": nc.dram_tensor("bdcn", [O, 1], f32, kind="ExternalInput").ap(),
        "cadd": nc.dram_tensor("cadd", [128, NI * CH], f32, kind="ExternalInput").ap(),
    }
    out_ap = nc.dram_tensor("out", [O, Q], f32, kind="ExternalOutput").ap()
    build_tile_kernel(nc, ins, out_ap)
    nc.compile()
    _cache["prog"] = nc
    return nc


def make_in_maps(x, w_off, b_off, w_dcn, b_dcn):
    woffp, woffs, wdcn_r, bdcn = _prep_weights(
        np.asarray(w_off), np.asarray(b_off), np.asarray(w_dcn), np.asarray(b_dcn)
    )
    cadd = _build_consts(np.asarray(b_off))
    x = np.asarray(x)
    in_maps = []
    for m in range(8):
        b, h = m // 2, m % 2
        xi = np.zeros((C, RH, RW), dtype=np.float32)
        r0 = h * NI - HALO
        rlo, rhi = max(0, -r0), min(RH, H - r0)
        xi[:, rlo:rhi, HALO : HALO + W] = x[b, :, r0 + rlo : r0 + rhi, :]
        in_maps.append(
            {
                "xi": np.ascontiguousarray(xi.reshape(C, NPIX)),
                "woffp": woffp,
                "woffs": woffs,
                "wdcn_r": wdcn_r,
                "bdcn": bdcn,
                "cadd": cadd,
            }
        )
    return in_maps


def kernel(x, w_off, b_off, w_dcn, b_dcn):
    from concourse import bass_utils

    nc = _get_program()
    in_maps = make_in_maps(x, w_off, b_off, w_dcn, b_dcn)
    res = bass_utils.run_bass_kernel_spmd(nc, in_maps, core_ids=list(range(8)))
    out = np.zeros((B, O, H, W), dtype=np.float32)
    for m in range(8):
        b, h = m // 2, m % 2
        out[b, :, h * NI : (h + 1) * NI, :] = res.results[m]["out"].reshape(O, NI, W)
    return out



# revision 19
# speedup vs baseline: 2.1372x; 1.0166x over previous
"""Deformable Conv2d (offset-conv -> bilinear sample -> 3x3 conv) on 8 NeuronCores.

Sharding: batch(4) x H-halves(2) -> 8 cores. Each core computes a [64, 64, 128]
slice of the output for one image. Inputs per core: a zero-padded halo slice of
its image plus (replicated) weights and index-offset constants.

Per-core device pipeline:
  1. offset conv (PE matmuls, fp16) -> offsets [18, 8192]
  2. transpose offsets to pixel-partitioned layout [128(j), 64(i), 18(ch)]
  3. index math on DVE: sampling positions, floor/frac, gather indices (int16),
     bilinear corner weight products (fp16)
  4. build a y-pair-expanded, channel-minor gather table in DRAM
     (cast to fp16 + PE transposes + 2 interleaved DMA writes)
  5. dma_gather (Pool/SWDGE): one 512B descriptor per (tap, output pixel)
     fetches all 4 bilinear corners for all 64 channels
  6. weight the gathered corners on DVE (per-pixel weights broadcast over
     channels via a step-0 free dim)
  7. PE transposes (PSUM-accumulated over the x-corner pair) to put (y-corner,
     channel) on partitions
  8. deform conv: PE matmuls contracting (y-corner, channel) per tap,
     accumulating the 9 taps in PSUM; bias via ACT on eviction.
"""

import numpy as np
from contextlib import ExitStack

B, C, H, W, O = 4, 64, 128, 128, 64
K2, CH = 9, 18
NI = 64               # output rows per core
HALO = 4
RH, RW = 72, 136      # halo slice dims (rows [h*64-4, h*64+68), cols [-4, 132))
NPIX = RH * RW        # 9792
TCH = 77              # ceil(NPIX/128) transpose chunks for the gather table
XHF = TCH * 128       # 9856 padded pixel count
TROWS = XHF           # gather-table rows (one per padded pixel)
Q = NI * W            # 8192 output pixels per core
ICH = 16              # i-rows per main-loop chunk
NCHUNK = NI // ICH    # 4 chunks
NIDX = ICH * W        # 2048 gather indices per (tap, chunk)
YCL = 70.99
XCL = 134.99

_cache = {}


def _ch_perm(ch):
    # offset-conv output channel order: ch in [0,9) -> oy of tap ch,
    # ch in [9,18) -> ox of tap ch-9. Source channel in w_off layout:
    return 2 * ch if ch < 9 else 2 * (ch - 9) + 1


def _build_consts(b_off):
    """Host-side constant tensors (identical for every core). The offset-conv
    bias is folded in here (cadd is added to the raw conv output)."""
    # cadd[j, i*18+ch]: base sampling position in halo-local coords + b_off
    cadd = np.zeros((128, NI, CH), dtype=np.float32)
    for chn in range(CH):
        if chn < 9:
            kh = chn // 3
            cadd[:, :, chn] = (np.arange(NI, dtype=np.float32) + 3 + kh)[None, :]
        else:
            kw = (chn - 9) % 3
            cadd[:, :, chn] = (np.arange(128, dtype=np.float32) + 3 + kw)[:, None]
        cadd[:, :, chn] += b_off[_ch_perm(chn)]
    return cadd.reshape(128, NI * CH)


def _prep_weights(w_off, b_off, w_dcn, b_dcn):
    # Offset-conv lhsT, packed for double-tap contraction: for each kernel row
    # kh, taps (kh,0) and (kh,1) contract together over K=128 (the image copy
    # on partitions 64-127 is pre-shifted one column), tap (kh,2) is a K=64
    # single. woffp[kh]: [128, 18]; woffs[kh]: [64, 18].
    woffp = np.zeros((3, 2 * C, CH), dtype=np.float32)
    woffs = np.zeros((3, C, CH), dtype=np.float32)
    for kh in range(3):
        for chn in range(CH):
            woffp[kh, :C, chn] = w_off[_ch_perm(chn), :, kh, 0]
            woffp[kh, C:, chn] = w_off[_ch_perm(chn), :, kh, 1]
            woffs[kh, :, chn] = w_off[_ch_perm(chn), :, kh, 2]
    # wdcn_r[k, a*64+c, o] : lhsT for deform conv tap k, replicated over the
    # y-corner index a (the transposed sampled tensor has (a, c) on partitions)
    wdcn_r = np.zeros((K2, 2 * C, O), dtype=np.float32)
    for k in range(K2):
        kh, kw = k // 3, k % 3
        wdcn_r[k, :C, :] = w_dcn[:, :, kh, kw].T
        wdcn_r[k, C:, :] = w_dcn[:, :, kh, kw].T
    return woffp, woffs, wdcn_r, b_dcn.reshape(O, 1).astype(np.float32)


def build_tile_kernel(nc, ins, out_ap, stage=99, repeat=1):
    """Emit the per-core program. ins: dict name -> AP (DRAM).
    stage truncates the pipeline for debugging (99 = full)."""
    import concourse.bass as bass
    import concourse.mybir as mybir
    import concourse.tile as tile
    from concourse.masks import make_identity

    f32 = mybir.dt.float32
    f16 = mybir.dt.float16
    i16 = mybir.dt.int16
    AF = mybir.ActivationFunctionType
    AO = mybir.AluOpType

    xi_d = ins["xi"]          # [64, NPIX] f32 halo slice
    woffp_d = ins["woffp"]    # [3, 128, 18] f32
    woffs_d = ins["woffs"]    # [3, 64, 18] f32
    wdcn_d = ins["wdcn_r"]    # [9, 128, 64] f32
    bdcn_d = ins["bdcn"]      # [64, 1] f32
    cadd_d = ins["cadd"]      # [128, 1152] f32

    tab_d = nc.dram_tensor("gtab", [(TROWS + 1) * 128], f16, kind="Internal")

    from concourse import library_config

    with ExitStack() as outer:
        tc = outer.enter_context(tile.TileContext(nc))
        nc.gpsimd.load_library(library_config.mlp)
        for _rep in range(repeat):
          with ExitStack() as ctx:
            consts = ctx.enter_context(tc.tile_pool(name="consts", bufs=1))
            sb = ctx.enter_context(tc.tile_pool(name="sb", bufs=1))
            setup_ctx = ctx.enter_context(ExitStack())
            ps_small = setup_ctx.enter_context(
                tc.tile_pool(name="ps_sm", bufs=2, space="PSUM")
            )

            # ---- constants in SBUF
            ident16 = consts.tile([128, 128], f16)
            make_identity(nc, ident16)
            ident32 = consts.tile([128, 128], f32)
            make_identity(nc, ident32)
            cadd_sb = consts.tile([128, NI * CH], f32)
            nc.sync.dma_start(cadd_sb[:], cadd_d[:])
            bdcn_sb = consts.tile([O, 1], f32)
            nc.sync.dma_start(bdcn_sb[:], bdcn_d[:])
            woffp32 = consts.tile([2 * C, 3 * CH], f32)
            nc.sync.dma_start(
                woffp32[:].rearrange("p (t c) -> p t c", t=3),
                woffp_d[:].rearrange("t p c -> p t c"),
            )
            woffph = consts.tile([2 * C, 3 * CH], f16)
            nc.vector.tensor_copy(woffph[:], woffp32[:])
            woffs32 = consts.tile([C, 3 * CH], f32)
            nc.sync.dma_start(
                woffs32[:].rearrange("p (t c) -> p t c", t=3),
                woffs_d[:].rearrange("t p c -> p t c"),
            )
            woffsh = consts.tile([C, 3 * CH], f16)
            nc.vector.tensor_copy(woffsh[:], woffs32[:])
            wdcn32 = consts.tile([128, K2 * O], f32)
            nc.sync.dma_start(
                wdcn32[:].rearrange("p (t c) -> p t c", t=K2),
                wdcn_d[:].rearrange("t p c -> p t c"),
            )
            wdcnh = consts.tile([128, K2 * O], f16)
            nc.vector.tensor_copy(wdcnh[:], wdcn32[:])

            # ---- load + cast x
            sbA = setup_ctx.enter_context(tc.tile_pool(name="sbA", bufs=1))
            sbB = setup_ctx.enter_context(tc.tile_pool(name="sbB", bufs=1))
            sbC = setup_ctx.enter_context(tc.tile_pool(name="sbC", bufs=1))
            xh = sbA.tile([128, XHF], f16)
            nc.vector.memset(xh[:, NPIX:], 0.0)
            with tc.tile_pool(name="xload", bufs=4) as xload:
                xcs = NPIX // 8  # 1224
                for t in range(8):
                    x32 = xload.tile([C, xcs], f32, tag="xc")
                    eng = nc.sync if t % 2 == 0 else nc.scalar
                    eng.dma_start(x32[:], xi_d[:, t * xcs : (t + 1) * xcs])
                    if t % 2 == 0:
                        nc.scalar.copy(xh[:C, t * xcs : (t + 1) * xcs], x32[:])
                    else:
                        nc.vector.tensor_copy(xh[:C, t * xcs : (t + 1) * xcs], x32[:])
            # partitions 64-127: same image shifted one column left (for the
            # double-tap conv contraction)
            nc.sync.dma_start(xh[C : 2 * C, 0 : NPIX - 1], xh[0:C, 1:NPIX])
            nc.vector.memset(xh[C : 2 * C, NPIX - 1 : NPIX], 0.0)

            # ---- gather-table build: transpose to pixel-major, interleave the
            # vertical pair (pixel p | pixel p+136) in SBUF, then one fat
            # DRAM write with 256B-contiguous runs per table row.
            xt = sbA.tile([128, TCH * 2 * C], f16)
            xt4 = xt[:].rearrange("p (t v c) -> p t v c", t=TCH, v=2)
            for t in range(TCH):
                pst = ps_small.tile([128, C], f16, tag="tabT")
                nc.tensor.transpose(
                    pst[:], xh[:C, t * 128 : (t + 1) * 128], ident16[:C, :C]
                )
                if t % 2 == 0:
                    nc.scalar.copy(xt4[:, t, 0, :], pst[:])
                else:
                    nc.vector.tensor_copy(xt4[:, t, 0, :], pst[:])
            # pair slot: xt4[p, t, 1, :] = pixel (t*128+p)+136 = xt4[p+8, t+1, 0, :]
            # (only rows < 9656 are ever gathered; tails can hold garbage)
            nc.scalar.dma_start(xt4[0:120, 0 : TCH - 1, 1, :], xt4[8:128, 1:TCH, 0, :])
            nc.sync.dma_start(xt4[120:128, 0 : TCH - 2, 1, :], xt4[0:8, 2:TCH, 0, :])
            TH = TCH // 2
            destA = bass.AP(
                tensor=tab_d, offset=0, ap=[[128, 128], [128 * 128, TH], [1, 2 * C]]
            )
            destB = bass.AP(
                tensor=tab_d,
                offset=TH * 128 * 128,
                ap=[[128, 128], [128 * 128, TCH - TH], [1, 2 * C]],
            )
            xt3f = xt4.rearrange("p t v c -> p t (v c)")
            nc.sync.dma_start(destA, xt3f[:, :TH, :])
            nc.scalar.dma_start(destB, xt3f[:, TH:, :])

            if stage < 2:
                return
            # ---- offset conv -> offs_sb [18, 8192] f32
            xh3 = xh[:, :NPIX].rearrange("p (r s) -> p r s", s=RW)
            offs_sb = sbB.tile([CH, Q], f16)
            with tc.tile_pool(name="ps_conv", bufs=3, space="PSUM") as ps_conv:
                for u in range(16):
                    psc = ps_conv.tile([CH, 512], f32, tag="conv")
                    for kh in range(3):
                        rows = slice(u * 4 + kh + 3, u * 4 + kh + 7)
                        nc.tensor.matmul(
                            psc[:],
                            woffph[:, kh * CH : (kh + 1) * CH],
                            xh3[:, rows, 3:131],
                            start=(kh == 0),
                            stop=False,
                        )
                        nc.tensor.matmul(
                            psc[:],
                            woffsh[:, kh * CH : (kh + 1) * CH],
                            xh3[:C, rows, 5:133],
                            start=False,
                            stop=(kh == 2),
                        )
                    if u % 2 == 0:
                        nc.scalar.copy(offs_sb[:, u * 512 : (u + 1) * 512], psc[:])
                    else:
                        nc.vector.tensor_copy(
                            offs_sb[:, u * 512 : (u + 1) * 512], psc[:]
                        )

            if stage < 3:
                return
            # ---- transpose offsets to [128(j), (i, ch)]
            offsT = sbC.tile([128, NI * CH], f32)
            for t in range(NI):
                pso = ps_small.tile([128, CH], f16, tag="offT")
                nc.tensor.transpose(
                    pso[:], offs_sb[:, t * 128 : (t + 1) * 128], ident16[:CH, :CH]
                )
                if t % 2 == 0:
                    nc.scalar.copy(offsT[:, t * CH : (t + 1) * CH], pso[:])
                else:
                    nc.vector.tensor_copy(offsT[:, t * CH : (t + 1) * CH], pso[:])

            # ---- index math (DVE) in [128, (i, ch)] layout
            pp = sbC.tile([128, NI * CH], f32)
            nc.vector.tensor_tensor(pp[:], offsT[:], cadd_sb[:], AO.add)
            nc.vector.tensor_scalar_max(pp[:], pp[:], 0.0)
            pp3 = pp[:].rearrange("p (i c) -> p i c", c=CH)
            nc.vector.tensor_scalar_min(pp3[:, :, 0:9], pp3[:, :, 0:9], YCL)
            nc.vector.tensor_scalar_min(pp3[:, :, 9:18], pp3[:, :, 9:18], XCL)
            # exact floor for 0 <= x < 2^22: magic-add rounds to nearest int,
            # then subtract 1 where the rounded value exceeds x
            MAGIC = float(1 << 23)
            fl = sbC.tile([128, NI * CH], f32)
            nc.vector.tensor_scalar(fl[:], pp[:], MAGIC, MAGIC, AO.add, AO.subtract)
            gt = sbC.tile([128, NI * CH], f32)
            nc.vector.tensor_tensor(gt[:], fl[:], pp[:], AO.is_gt)
            nc.vector.tensor_tensor(fl[:], fl[:], gt[:], AO.subtract)
            fr = gt  # reuse
            nc.vector.tensor_tensor(fr[:], pp[:], fl[:], AO.subtract)
            fl3 = fl[:].rearrange("p (i c) -> p i c", c=CH)
            idxf = sbC.tile([128, NI * K2], f32)
            idxf3 = idxf[:].rearrange("p (k i) -> p i k", i=NI)
            nc.vector.scalar_tensor_tensor(
                idxf3, fl3[:, :, 0:9], 136.0, fl3[:, :, 9:18], AO.mult, AO.add
            )
            idx16 = sb.tile([128, NI * K2], i16)
            nc.vector.tensor_copy(idx16[:], idxf[:])
            wm1 = sbC.tile([128, NI * CH], f32)
            nc.vector.tensor_scalar(wm1[:], fr[:], -1.0, 1.0, AO.mult, AO.add)
            fr3 = fr[:].rearrange("p (i c) -> p i c", c=CH)
            wm13 = wm1[:].rearrange("p (i c) -> p i c", c=CH)
            wp = sb.tile([128, NI * K2 * 4], f16)
            wp5 = wp[:].rearrange("p (i k b a) -> p i k b a", k=K2, b=2, a=2)
            for b in range(2):
                wx = fr3[:, :, 9:18] if b else wm13[:, :, 9:18]
                for a in range(2):
                    wy = fr3[:, :, 0:9] if a else wm13[:, :, 0:9]
                    nc.vector.tensor_tensor(wp5[:, :, :, b, a], wx, wy, AO.mult)

            # ---- wrap indices for dma_gather: [16, f] replicated over 8 groups.
            # Partition fold 128->16 via 8 bulk partition-shift DMAs into a
            # (jj, k, i) staging tile, then a per-tap DVE free-dim permute to
            # (k, i, jj); replication to 128 partitions by doubling DMAs.
            idxw = sb.tile([128, K2 * (Q // 16)], i16)
            idxw3 = idxw[:].rearrange("p (k f) -> p k f", k=K2)
            idx163 = idx16[:].rearrange("p (k i) -> p k i", i=NI)
            idxw4 = idxw3[:, :, :].rearrange("p k (i j) -> p k i j", j=8)
            tmpw = sb.tile([16, 8 * K2 * NI], i16)
            tmp3 = tmpw[:].rearrange("p (j k i) -> p j k i", j=8, k=K2)
            for jj in range(8):
                eng = nc.sync if jj % 2 == 0 else nc.scalar
                eng.dma_start(tmp3[:, jj, :, :], idx163[16 * jj : 16 * jj + 16, :, :])
            for k in range(K2):
                nc.vector.tensor_copy(
                    idxw4[0:16, k, :, :],
                    tmp3[:, :, k, :].rearrange("p j i -> p i j"),
                )
                for gi, g in enumerate((16, 32, 64)):
                    eng = nc.sync if gi % 2 == 0 else nc.scalar
                    eng.dma_start(idxw3[g : 2 * g, k, :], idxw3[0:g, k, :])

            if stage == 35:
                # debug: dump idxf and a roundtripped idx16 into the output
                idxchk = sb.tile([128, NI * K2], f32)
                nc.vector.tensor_copy(idxchk[:], idx16[:])
                d0 = bass.AP(tensor=out_ap.tensor, offset=0, ap=[[576, 128], [1, 576]])
                nc.sync.dma_start(d0, idxf[:])
                d1 = bass.AP(
                    tensor=out_ap.tensor, offset=128 * 576, ap=[[576, 128], [1, 576]]
                )
                nc.sync.dma_start(d1, idxchk[:])
                d2 = bass.AP(
                    tensor=out_ap.tensor, offset=2 * 128 * 576, ap=[[1152, 128], [1, 1152]]
                )
                nc.sync.dma_start(d2, fr[:])
                return
            if stage < 4:
                return
            # ---- main loop: gather -> weight -> transpose -> per-tap deform
            # matmul accumulated in PSUM (no sacc staging buffer).
            setup_ctx.close()
            pmain = ctx.enter_context(tc.tile_pool(name="pmain", bufs=8))
            vspool = ctx.enter_context(tc.tile_pool(name="vspool", bufs=3))
            stpool = ctx.enter_context(tc.tile_pool(name="stpool", bufs=3))
            ps_t = ctx.enter_context(tc.tile_pool(name="ps_t", bufs=2, space="PSUM"))
            ps_o = ctx.enter_context(tc.tile_pool(name="ps_o", bufs=1, space="PSUM"))
            obp = ctx.enter_context(tc.tile_pool(name="ob", bufs=2))
            gsrc = bass.AP(tensor=tab_d, offset=0, ap=[[128, TROWS], [1, 256]])
            nchunk_run = NCHUNK if stage >= 43 else 1
            ntap_run = K2 if stage != 41 else 1
            for u in range(nchunk_run):
                psos = []
                for w in range(NIDX // 512):
                    pso_w = ps_o.tile([O, 512], f32, tag=f"out{w}", name=f"pso{w}")
                    psos.append(pso_w)
                for k in range(ntap_run):
                    v = pmain.tile([128, ICH * 256], f16, tag="V")
                    v3 = v[:].rearrange("p (i e) -> p i e", e=256)
                    # two 1024-idx halves on different queues: smaller ring
                    # footprint avoids Pool head-of-line blocking on ring-full
                    for h in range(2):
                        nc.gpsimd.dma_gather(
                            v3[:, h * (ICH // 2) : (h + 1) * (ICH // 2), :],
                            gsrc,
                            idxw3[
                                :,
                                k,
                                u * (NIDX // 16) + h * (NIDX // 32) : u * (NIDX // 16)
                                + (h + 1) * (NIDX // 32),
                            ],
                            num_idxs=NIDX // 2,
                            num_idxs_reg=NIDX // 2,
                            elem_size=256,
                            elem_step=128,
                            transpose=False,
                            single_packet=False,
                            queue_num=(u * K2 * 2 + k * 2 + h) % 4,
                        )
                    if stage < 5:
                        continue
                    v5 = v[:].rearrange("p (i b a c) -> p i b a c", i=ICH, b=2, a=2)
                    wslice = wp5[:, u * ICH : (u + 1) * ICH, k, :, :].broadcast_to(
                        [128, ICH, 2, 2, C]
                    )
                    nc.vector.tensor_tensor(v5, v5, wslice, AO.mult)
                    # sum the x-corner pair (b) -> [128, (i, a, c)]
                    vs = vspool.tile([128, ICH * 128], f16, tag="VS")
                    vs3 = vs[:].rearrange("p (i e) -> p i e", e=128)
                    nc.vector.tensor_tensor(
                        vs3, v5[:, :, 0, :, :], v5[:, :, 1, :, :], AO.add
                    )
                    pt = ps_t.tile([128, ICH * 128], f16, tag="T")
                    for i in range(ICH):
                        nc.tensor.matmul(
                            pt[:, i * 128 : (i + 1) * 128],
                            vs3[:, i, :],
                            ident16,
                            is_transpose=True,
                            start=True,
                            stop=True,
                        )
                    if stage < 6:
                        continue
                    stap = stpool.tile([128, ICH * 128], f16, tag="ST")
                    nc.scalar.copy(stap[:], pt[:])
                    for w in range(NIDX // 512):
                        nc.tensor.matmul(
                            psos[w][:],
                            wdcnh[:, k * O : (k + 1) * O],
                            stap[:, w * 512 : (w + 1) * 512],
                            start=(k == 0),
                            stop=(k == 8),
                        )
                if stage < 6:
                    continue
                for w in range(NIDX // 512):
                    ob = obp.tile([O, 512], f32, tag="ob")
                    nc.vector.tensor_scalar_add(ob[:], psos[w][:], bdcn_sb[:])
                    nc.sync.dma_start(
                        out_ap[:, u * NIDX + w * 512 : u * NIDX + (w + 1) * 512], ob[:]
                    )


def _get_program():
    if "prog" in _cache:
        return _cache["prog"]
    import concourse.bacc as bacc
    import concourse.mybir as mybir

    f32 = mybir.dt.float32
    nc = bacc.Bacc(
        "TRN2",
        target_bir_lowering=False,
        debug=False,
        num_devices=8,
        num_swdge_queues=4,
    )
    ins = {
        "xi": nc.dram_tensor("xi", [C, NPIX], f32, kind="ExternalInput").ap(),
        "woffp": nc.dram_tensor("woffp", [3, 2 * C, CH], f32, kind="ExternalInput").ap(),
        "woffs": nc.dram_tensor("woffs", [3, C, CH], f32, kind="ExternalInput").ap(),
        "wdcn_r": nc.dram_tensor("wdcn_r", [K2, 2 * C, O], f32, kind="ExternalInput").ap(),
        "bdcn": nc.dram_tensor("bdcn", [O, 1], f32, kind="ExternalInput").ap(),
        "cadd": nc.dram_tensor("cadd", [128, NI * CH], f32, kind="ExternalInput").ap(),
    }
    out_ap = nc.dram_tensor("out", [O, Q], f32, kind="ExternalOutput").ap()
    build_tile_kernel(nc, ins, out_ap)
    nc.compile()
    _cache["prog"] = nc
    return nc


def make_in_maps(x, w_off, b_off, w_dcn, b_dcn):
    woffp, woffs, wdcn_r, bdcn = _prep_weights(
        np.asarray(w_off), np.asarray(b_off), np.asarray(w_dcn), np.asarray(b_dcn)
    )
    cadd = _build_consts(np.asarray(b_off))
    x = np.asarray(x)
    in_maps = []
    for m in range(8):
        b, h = m // 2, m % 2
        xi = np.zeros((C, RH, RW), dtype=np.float32)
        r0 = h * NI - HALO
        rlo, rhi = max(0, -r0), min(RH, H - r0)
        xi[:, rlo:rhi, HALO : HALO + W] = x[b, :, r0 + rlo : r0 + rhi, :]
        in_maps.append(
            {
                "xi": np.ascontiguousarray(xi.reshape(C, NPIX)),
                "woffp": woffp,
                "woffs": woffs,
                "wdcn_r": wdcn_r,
                "bdcn": bdcn,
                "cadd": cadd,
            }
        )
    return in_maps


def kernel(x, w_off, b_off, w_dcn, b_dcn):
    from concourse import bass_utils

    nc = _get_program()
    in_maps = make_in_maps(x, w_off, b_off, w_dcn, b_dcn)
    res = bass_utils.run_bass_kernel_spmd(nc, in_maps, core_ids=list(range(8)))
    out = np.zeros((B, O, H, W), dtype=np.float32)
    for m in range(8):
        b, h = m // 2, m % 2
        out[b, :, h * NI : (h + 1) * NI, :] = res.results[m]["out"].reshape(O, NI, W)
    return out

